# revision 9
# baseline (speedup 1.0000x reference)
"""GATv2Conv (heads=4, concat=False, self-loops) + GraphNorm on 8 TRN2 NeuronCores.

Sharding: destination nodes are split into 8 contiguous ranges (6250 each).
Each core computes xl/xr for its own node range, all-gathers xl (gather
source table), then processes its incoming edges sorted by destination in
125-dst windows. Per 128-edge chunk: dma_gather of xl[src]/xr[dst] rows,
attention logits via fused DVE ops, unnormalized softmax weights, and a
one-hot matmul that segment-reduces both the softmax denominators and the
weighted feature sums into PSUM. Window epilogue normalizes, head-means and
accumulates GraphNorm statistics; a final AllReduce + affine pass applies
GraphNorm.
"""
import sys

sys.path.insert(0, "/opt/trn_rl_repo")

import numpy as np
from concourse import bacc, mybir, tile
from concourse.bass_utils import run_bass_kernel_spmd
from concourse.masks import make_identity

N = 50000
NCORES = 8
NSH = N // NCORES          # 6250 dst nodes per core
IN_F = 256
H = 4
C = 64
F = H * C                  # 256
W = 125                    # dst window size
NW = NSH // W              # 50 windows per core
HALF = N // 2              # gather table half size (int16 index limit)
NEG = 0.2
EPS = 1e-5
MASKVAL = -100.0           # logit bias for padding edges -> exp == 0
CHUNK = 128
GRP = 4                    # chunks fused per DVE op group

f32 = mybir.dt.float32
i16 = mybir.dt.int16

LAST_RESULTS = None


def _pack_idx(idx: np.ndarray) -> np.ndarray:
    """[n] int -> [128, n//16] int16 gather-index layout (16-partition wrap,
    replicated for the 8 Q7 cores)."""
    n = idx.shape[0]
    pk = np.zeros((16, n // 16), np.int16)
    pk[np.arange(n) % 16, np.arange(n) // 16] = idx.astype(np.int16)
    return np.tile(pk, (8, 1))


def _prep_edges(src: np.ndarray, dst: np.ndarray):
    """Partition/sort/pad edges. Returns (cl, ch, per_core) where cl/ch are
    per-window chunk counts (shared across cores) and per_core is a list of
    dicts with IDX (int16 [128, icols]) and RM (f32 [128, rmcols])."""
    src = src.astype(np.int64)
    dst = dst.astype(np.int64)
    core = dst // NSH
    per_core_raw = []
    nlo = np.zeros((NCORES, NW), np.int64)
    nhi = np.zeros((NCORES, NW), np.int64)
    for c in range(NCORES):
        m = core == c
        s = src[m]
        d = dst[m] - c * NSH
        win = d // W
        half = s // HALF
        order = np.lexsort((half, win))
        s, d, win, half = s[order], d[order], win[order], half[order]
        key = win * 2 + half
        cnt = np.bincount(key, minlength=NW * 2).reshape(NW, 2)
        nlo[c] = cnt[:, 0]
        nhi[c] = cnt[:, 1]
        per_core_raw.append((s, d, np.cumsum(cnt.reshape(-1))))
    cl = np.ceil(nlo.max(axis=0) / CHUNK).astype(int)
    ch = np.ceil(nhi.max(axis=0) / CHUNK).astype(int)

    per_core = []
    for c in range(NCORES):
        s, d, cum = per_core_raw[c]
        idx_cols = []
        rm_cols = []
        for w in range(NW):
            beg_lo = cum[w * 2 - 1] if w * 2 > 0 else 0
            end_lo = cum[w * 2]
            end_hi = cum[w * 2 + 1]
            s_lo, d_lo = s[beg_lo:end_lo], d[beg_lo:end_lo]
            s_hi, d_hi = s[end_lo:end_hi], d[end_lo:end_hi]
            nL, nH = cl[w] * CHUNK, ch[w] * CHUNK
            padL = nL - len(s_lo)
            padH = nH - len(s_hi)
            a_lo = np.concatenate([s_lo, np.zeros(padL, np.int64)])
            a_hi = np.concatenate([s_hi - HALF, np.zeros(padH, np.int64)])
            dd = np.concatenate(
                [d_lo, np.full(padL, w * W, np.int64),
                 d_hi, np.full(padH, w * W, np.int64)]
            )
            rr = np.concatenate(
                [d_lo - w * W, np.zeros(padL, np.int64),
                 d_hi - w * W, np.zeros(padH, np.int64)]
            ).astype(np.float32)
            mm = np.concatenate(
                [np.zeros(len(s_lo), np.float32), np.full(padL, MASKVAL, np.float32),
                 np.zeros(len(s_hi), np.float32), np.full(padH, MASKVAL, np.float32)]
            )
            cols = []
            if nL:
                cols.append(_pack_idx(a_lo))
            if nH:
                cols.append(_pack_idx(a_hi))
            cols.append(_pack_idx(dd))
            idx_cols.append(np.concatenate(cols, axis=1))
            cpw = cl[w] + ch[w]
            rm = np.empty((128, 2 * cpw), np.float32)
            rm[:, :cpw] = rr.reshape(cpw, CHUNK).T
            rm[:, cpw:] = mm.reshape(cpw, CHUNK).T
            rm_cols.append(rm)
        per_core.append(
            dict(
                IDX=np.concatenate(idx_cols, axis=1),
                RM=np.concatenate(rm_cols, axis=1),
            )
        )
    return cl, ch, per_core


def _build(cl, ch, icols, rmcols):
    """Build the 8-core SPMD graph. cl/ch: per-window chunk counts."""
    import os
    bisect = int(os.environ.get("BISECT", "0"))
    nwin = int(os.environ.get("NWIN", str(NW)))
    nc = bacc.Bacc(None, target_bir_lowering=False, debug=False, num_devices=NCORES)

    xs_in = nc.declare_dram_parameter("XS", [NSH, IN_F], f32, isOutput=False)
    wl_in = nc.declare_dram_parameter("WL", [IN_F, F], f32, isOutput=False)
    wr_in = nc.declare_dram_parameter("WR", [IN_F, F], f32, isOutput=False)
    attb_in = nc.declare_dram_parameter("ATTB", [128, F], f32, isOutput=False)
    gnp_in = nc.declare_dram_parameter("GNP", [C, 4], f32, isOutput=False)
    diota_in = nc.declare_dram_parameter("DIOTA", [128, W], f32, isOutput=False)
    idx_in = nc.declare_dram_parameter("IDX", [128, icols], i16, isOutput=False)
    rm_in = nc.declare_dram_parameter("RM", [128, rmcols], f32, isOutput=False)
    out_ext = nc.declare_dram_parameter("OUT", [NSH, C], f32, isOutput=True)

    xl_sh = nc.dram_tensor("xl_sh", [NSH, F], f32)
    xr_d = nc.dram_tensor("xr_d", [NSH, F], f32)
    xl_full = nc.dram_tensor("xl_full", [N, F], f32, addr_space="Shared")
    om_d = nc.dram_tensor("om_d", [NSH, C], f32)
    stats_l = nc.dram_tensor("stats_l", [C, 2], f32)
    stats_g = nc.dram_tensor("stats_g", [C, 2], f32, addr_space="Shared")

    cpw = [int(cl[w] + ch[w]) for w in range(NW)]
    cpw_max = max(cpw)
    ioff = np.concatenate([[0], np.cumsum([2 * cpw[w] * 8 for w in range(NW)])])
    roff = np.concatenate([[0], np.cumsum([2 * cpw[w] for w in range(NW)])])

    with tile.TileContext(nc) as tc:
        with (
            tc.tile_pool(name="const", bufs=1) as cp,
            tc.tile_pool(name="sb", bufs=2) as sb,
            tc.tile_pool(name="acc", bufs=1, space="PSUM") as accp,
        ):
            ident = cp.tile([128, 128], f32)
            make_identity(nc, ident[:])
            wl_t = cp.tile([128, 2, F], f32)
            wr_t = cp.tile([128, 2, F], f32)
            nc.sync.dma_start(wl_t[:], wl_in.ap().rearrange("(s k) n -> k s n", k=128))
            nc.sync.dma_start(wr_t[:], wr_in.ap().rearrange("(s k) n -> k s n", k=128))
            attb_t = cp.tile([128, F], f32)
            nc.sync.dma_start(attb_t[:], attb_in[:, :])
            gnp_t = cp.tile([128, 4], f32)
            nc.sync.dma_start(gnp_t[:C, :], gnp_in[:, :])
            diota_t = cp.tile([128, W], f32)
            nc.sync.dma_start(diota_t[:], diota_in[:, :])
            ones_n = cp.tile([128, 1], f32)
            nc.vector.memset(ones_n[:], 1.0)
            ones_r = cp.tile([128, W], f32)
            nc.vector.memset(ones_r[0:1, :], 1.0)
            biasb_t = cp.tile([128, C], f32)
            sb_t = cp.tile([128, C], f32)
            tb_t = cp.tile([128, C], f32)

            # biasB: [W, C] replicated GAT bias (gnp col 3)
            with tc.tile_pool(name="psi", bufs=1, space="PSUM") as psi:
                brow_ps = psi.tile([128, C], f32, space="PSUM", tag="brow")
                nc.tensor.transpose(brow_ps[0:1, :], gnp_t[:C, 3:4], ident[:C, :C])
                brow_t = cp.tile([128, C], f32)
                nc.vector.tensor_copy(out=brow_t[0:1, :], in_=brow_ps[0:1, :])
                biasb_ps = psi.tile([128, C], f32, space="PSUM", tag="bb")
                nc.tensor.matmul(
                    biasb_ps[:W, :], lhsT=ones_r[0:1, :], rhs=brow_t[0:1, :],
                    start=True, stop=True,
                )
                nc.vector.tensor_copy(out=biasb_t[:W, :], in_=biasb_ps[:W, :])

            # ---- P0: xl = X @ Wl, xr = X @ Wr for own shard ----
            ntile = (NSH + 127) // 128
            with tc.tile_pool(name="ps0", bufs=2, space="PSUM") as ps0:
                for i in range(ntile):
                    rows = min(128, NSH - i * 128)
                    x_t = sb.tile([128, IN_F], f32, tag="p0x")
                    nc.sync.dma_start(x_t[:rows, :], xs_in[i * 128:i * 128 + rows, :])
                    xt_sb = sb.tile([128, 2, 128], f32, tag="p0xt")
                    for hh in range(2):
                        xt_ps = ps0.tile([128, 128], f32, space="PSUM", tag="p0tp")
                        nc.tensor.transpose(
                            xt_ps[:, :rows], x_t[:rows, hh * 128:(hh + 1) * 128],
                            ident[:rows, :rows],
                        )
                        nc.vector.tensor_copy(out=xt_sb[:, hh, :rows], in_=xt_ps[:, :rows])
                    for w_t, dram in ((wl_t, xl_sh), (wr_t, xr_d)):
                        mm_ps = ps0.tile([128, F], f32, space="PSUM", tag="p0mm")
                        for hh in range(2):
                            nc.tensor.matmul(
                                mm_ps[:rows, :], lhsT=xt_sb[:, hh, :rows],
                                rhs=w_t[:, hh, :], start=(hh == 0), stop=(hh == 1),
                            )
                        o_t = sb.tile([128, F], f32, tag="p0o")
                        nc.vector.tensor_copy(out=o_t[:rows, :], in_=mm_ps[:rows, :])
                        nc.sync.dma_start(dram[i * 128:i * 128 + rows, :], o_t[:rows, :])

            # ---- P1: all-gather xl ----
            nc.gpsimd.collective_compute(
                "AllGather", mybir.AluOpType.bypass,
                replica_groups=[list(range(NCORES))],
                ins=[xl_sh.ap().opt()], outs=[xl_full.ap().opt()],
            )

            # ---- P2: windowed edge processing ----
            if bisect == 1:
                for w in range(NW):
                    z_t = sb.tile([128, C], f32, tag="z")
                    nc.sync.dma_start(z_t[:W, :], xl_full[w * W:(w + 1) * W, 0:C])
                    nc.sync.dma_start(out_ext[w * W:(w + 1) * W, :], z_t[:W, :])
                nc.compile()
                return nc
            sum_ps = accp.tile([128, 1], f32, space="PSUM", tag="accsum")
            ssq_ps = accp.tile([128, 1], f32, space="PSUM", tag="accssq")
            with tc.tile_pool(name="ps2", bufs=2, space="PSUM") as ps2:
                for w in range(nwin):
                    nL, nH, nT = int(cl[w]), int(ch[w]), cpw[w]
                    ix_t = sb.tile([128, (2 * cpw_max) * 8], i16, tag="ix")
                    nc.sync.dma_start(
                        ix_t[:, :2 * nT * 8], idx_in[:, int(ioff[w]):int(ioff[w + 1])]
                    )
                    rm_t = sb.tile([128, 2 * cpw_max], f32, tag="rm")
                    nc.sync.dma_start(
                        rm_t[:, :2 * nT], rm_in[:, int(roff[w]):int(roff[w + 1])]
                    )
                    a_t = sb.tile([128, cpw_max, F], f32, tag="a")
                    b_t = sb.tile([128, cpw_max, F], f32, tag="b")
                    if nL:
                        nc.gpsimd.dma_gather(
                            a_t[:, :nL, :], xl_full[0:HALF, :], ix_t[:, :nL * 8],
                            nL * CHUNK, nL * CHUNK, F,
                            single_packet=(nL * CHUNK <= 1024),
                        )
                    if nH:
                        nc.gpsimd.dma_gather(
                            a_t[:, nL:nT, :], xl_full[HALF:N, :],
                            ix_t[:, nL * 8:nT * 8], nH * CHUNK, nH * CHUNK, F,
                            single_packet=(nH * CHUNK <= 1024),
                        )
                    nc.gpsimd.dma_gather(
                        b_t[:, :nT, :], xr_d[:, :], ix_t[:, nT * 8:2 * nT * 8],
                        nT * CHUNK, nT * CHUNK, F,
                        single_packet=(nT * CHUNK <= 1024),
                    )
                    den_ps = ps2.tile([W, H], f32, space="PSUM", tag="den")
                    out_ps = ps2.tile([W, F], f32, space="PSUM", tag="outp")
                    p_buf = sb.tile([128, cpw_max, H], f32, tag="p")
                    for k0 in range(0, nT, GRP):
                        g = min(GRP, nT - k0)
                        s4 = sb.tile([128, GRP, F], f32, tag="s4")
                        nc.vector.tensor_tensor(
                            out=s4[:, :g, :], in0=a_t[:, k0:k0 + g, :],
                            in1=b_t[:, k0:k0 + g, :], op=mybir.AluOpType.add,
                        )
                        l4 = sb.tile([128, GRP, F], f32, tag="l4")
                        nc.vector.scalar_tensor_tensor(
                            out=l4[:, :g, :], in0=s4[:, :g, :], scalar=NEG,
                            in1=s4[:, :g, :], op0=mybir.AluOpType.mult,
                            op1=mybir.AluOpType.max,
                        )
                        t4 = sb.tile([128, GRP, F], f32, tag="t4")
                        nc.vector.tensor_tensor(
                            out=t4[:, :g, :], in0=l4[:, :g, :],
                            in1=attb_t[:].rearrange("p (o f) -> p o f", o=1).to_broadcast([128, g, F]),
                            op=mybir.AluOpType.mult,
                        )
                        lg4 = sb.tile([128, GRP, H], f32, tag="lg4")
                        nc.vector.reduce_sum(
                            out=lg4[:, :g, :],
                            in_=t4[:, :g, :].rearrange("p k (h c) -> p k h c", h=H),
                            axis=mybir.AxisListType.X,
                        )
                        for j in range(g):
                            nc.scalar.activation(
                                p_buf[:, k0 + j, :], lg4[:, j, :],
                                mybir.ActivationFunctionType.Exp,
                                bias=rm_t[:, nT + k0 + j:nT + k0 + j + 1], scale=1.0,
                            )
                        m014 = sb.tile([128, GRP, W], f32, tag="m014")
                        nc.vector.tensor_tensor(
                            out=m014[:, :g, :],
                            in0=diota_t[:].rearrange("p (o w) -> p o w", o=1).to_broadcast([128, g, W]),
                            in1=rm_t[:, k0:k0 + g].rearrange("p (k o) -> p k o", o=1).to_broadcast([128, g, W]),
                            op=mybir.AluOpType.is_equal,
                        )
                        ap4 = sb.tile([128, GRP, H, C], f32, tag="ap4")
                        nc.vector.tensor_tensor(
                            out=ap4[:, :g, :, :],
                            in0=a_t[:, k0:k0 + g, :].rearrange("p k (h c) -> p k h c", h=H),
                            in1=p_buf[:, k0:k0 + g, :].rearrange("p k (h o) -> p k h o", o=1).to_broadcast([128, g, H, C]),
                            op=mybir.AluOpType.mult,
                        )
                        for j in range(g):
                            k = k0 + j
                            nc.tensor.matmul(
                                den_ps[:], lhsT=m014[:, j, :], rhs=p_buf[:, k, :],
                                start=(k == 0), stop=(k == nT - 1),
                            )
                            nc.tensor.matmul(
                                out_ps[:], lhsT=m014[:, j, :], rhs=ap4[:, j, :, :],
                                start=(k == 0), stop=(k == nT - 1),
                            )
                    # window epilogue
                    rd_t = sb.tile([128, H], f32, tag="rd")
                    nc.vector.reciprocal(rd_t[:W, :], den_ps[:])
                    oh_t = sb.tile([128, H, C], f32, tag="oh")
                    for h in range(H):
                        nc.vector.tensor_scalar(
                            out=oh_t[:W, h, :], in0=out_ps[:, h * C:(h + 1) * C],
                            scalar1=rd_t[:W, h:h + 1], scalar2=0.25,
                            op0=mybir.AluOpType.mult, op1=mybir.AluOpType.mult,
                        )
                    o01 = sb.tile([128, C], f32, tag="o01")
                    nc.vector.tensor_tensor(
                        out=o01[:W, :], in0=oh_t[:W, 0, :], in1=oh_t[:W, 1, :],
                        op=mybir.AluOpType.add,
                    )
                    o23 = sb.tile([128, C], f32, tag="o23")
                    nc.vector.tensor_tensor(
                        out=o23[:W, :], in0=oh_t[:W, 2, :], in1=oh_t[:W, 3, :],
                        op=mybir.AluOpType.add,
                    )
                    o0123 = sb.tile([128, C], f32, tag="o0123")
                    nc.vector.tensor_tensor(
                        out=o0123[:W, :], in0=o01[:W, :], in1=o23[:W, :],
                        op=mybir.AluOpType.add,
                    )
                    om_t = sb.tile([128, C], f32, tag="om")
                    nc.vector.tensor_tensor(
                        out=om_t[:W, :], in0=o0123[:W, :], in1=biasb_t[:W, :],
                        op=mybir.AluOpType.add,
                    )
                    sq_t = sb.tile([128, C], f32, tag="sq")
                    nc.scalar.square(sq_t[:W, :], om_t[:W, :])
                    nc.tensor.matmul(
                        sum_ps[:C, :], lhsT=om_t[:W, :], rhs=ones_n[:W, :],
                        start=(w == 0), stop=(w == nwin - 1),
                    )
                    nc.tensor.matmul(
                        ssq_ps[:C, :], lhsT=sq_t[:W, :], rhs=ones_n[:W, :],
                        start=(w == 0), stop=(w == nwin - 1),
                    )
                    nc.sync.dma_start(om_d[w * W:(w + 1) * W, :], om_t[:W, :])

            if bisect == 2:
                for w in range(NW):
                    z_t = sb.tile([128, C], f32, tag="z")
                    nc.sync.dma_start(z_t[:W, :], om_d[w * W:(w + 1) * W, :])
                    nc.sync.dma_start(out_ext[w * W:(w + 1) * W, :], z_t[:W, :])
                nc.compile()
                return nc
            # ---- P3: GraphNorm statistics ----
            st_t = sb.tile([128, 2], f32, tag="st")
            nc.vector.tensor_copy(out=st_t[:C, 0:1], in_=sum_ps[:C, :])
            nc.vector.tensor_copy(out=st_t[:C, 1:2], in_=ssq_ps[:C, :])
            nc.sync.dma_start(stats_l[:, :], st_t[:C, :])
            nc.gpsimd.collective_compute(
                "AllReduce", mybir.AluOpType.add,
                replica_groups=[list(range(NCORES))],
                ins=[stats_l.ap().opt()], outs=[stats_g.ap().opt()],
            )
            sg_t = sb.tile([128, 2], f32, tag="sg")
            nc.sync.dma_start(sg_t[:C, :], stats_g[:, :])
            mu_t = sb.tile([128, 1], f32, tag="mu")
            nc.vector.tensor_scalar_mul(mu_t[:C, :], sg_t[:C, 0:1], 1.0 / N)
            msq_t = sb.tile([128, 1], f32, tag="msq")
            nc.vector.tensor_scalar_mul(msq_t[:C, :], sg_t[:C, 1:2], 1.0 / N)
            amu_t = sb.tile([128, 1], f32, tag="amu")
            nc.vector.tensor_tensor(
                out=amu_t[:C, :], in0=gnp_t[:C, 2:3], in1=mu_t[:C, :],
                op=mybir.AluOpType.mult,
            )
            am2_t = sb.tile([128, 1], f32, tag="am2")
            nc.vector.scalar_tensor_tensor(
                out=am2_t[:C, :], in0=mu_t[:C, :], scalar=-2.0, in1=amu_t[:C, :],
                op0=mybir.AluOpType.mult, op1=mybir.AluOpType.add,
            )
            var_t = sb.tile([128, 1], f32, tag="var")
            nc.vector.tensor_tensor(
                out=var_t[:C, :], in0=amu_t[:C, :], in1=am2_t[:C, :],
                op=mybir.AluOpType.mult,
            )
            nc.vector.tensor_tensor(
                out=var_t[:C, :], in0=var_t[:C, :], in1=msq_t[:C, :],
                op=mybir.AluOpType.add,
            )
            nc.vector.tensor_scalar_add(var_t[:C, :], var_t[:C, :], EPS)
            sd_t = sb.tile([128, 1], f32, tag="sd")
            nc.scalar.sqrt(sd_t[:C, :], var_t[:C, :])
            inv_t = sb.tile([128, 1], f32, tag="inv")
            nc.vector.reciprocal(inv_t[:C, :], sd_t[:C, :])
            st2_t = sb.tile([128, 2], f32, tag="st2")
            nc.vector.tensor_tensor(
                out=st2_t[:C, 0:1], in0=gnp_t[:C, 0:1], in1=inv_t[:C, :],
                op=mybir.AluOpType.mult,
            )
            u_t = sb.tile([128, 1], f32, tag="u")
            nc.vector.tensor_tensor(
                out=u_t[:C, :], in0=st2_t[:C, 0:1], in1=amu_t[:C, :],
                op=mybir.AluOpType.mult,
            )
            nc.vector.tensor_tensor(
                out=st2_t[:C, 1:2], in0=gnp_t[:C, 1:2], in1=u_t[:C, :],
                op=mybir.AluOpType.subtract,
            )
            with tc.tile_pool(name="ps3", bufs=1, space="PSUM") as ps3:
                srow_ps = ps3.tile([128, C], f32, space="PSUM", tag="srow")
                nc.tensor.transpose(srow_ps[0:1, :], st2_t[:C, 0:1], ident[:C, :C])
                trow_ps = ps3.tile([128, C], f32, space="PSUM", tag="trow")
                nc.tensor.transpose(trow_ps[0:1, :], st2_t[:C, 1:2], ident[:C, :C])
                srow_t = sb.tile([128, C], f32, tag="srow")
                nc.vector.tensor_copy(out=srow_t[0:1, :], in_=srow_ps[0:1, :])
                trow_t = sb.tile([128, C], f32, tag="trow")
                nc.vector.tensor_copy(out=trow_t[0:1, :], in_=trow_ps[0:1, :])
                sb_ps = ps3.tile([128, C], f32, space="PSUM", tag="sbp")
                nc.tensor.matmul(
                    sb_ps[:W, :], lhsT=ones_r[0:1, :], rhs=srow_t[0:1, :],
                    start=True, stop=True,
                )
                nc.vector.tensor_copy(out=sb_t[:W, :], in_=sb_ps[:W, :])
                tb_ps = ps3.tile([128, C], f32, space="PSUM", tag="tbp")
                nc.tensor.matmul(
                    tb_ps[:W, :], lhsT=ones_r[0:1, :], rhs=trow_t[0:1, :],
                    start=True, stop=True,
                )
                nc.vector.tensor_copy(out=tb_t[:W, :], in_=tb_ps[:W, :])

            # ---- P4: apply GraphNorm affine ----
            for w in range(NW):
                omw_t = sb.tile([128, C], f32, tag="omw")
                nc.sync.dma_start(omw_t[:W, :], om_d[w * W:(w + 1) * W, :])
                y_t = sb.tile([128, C], f32, tag="y")
                nc.vector.tensor_tensor(
                    out=y_t[:W, :], in0=omw_t[:W, :], in1=sb_t[:W, :],
                    op=mybir.AluOpType.mult,
                )
                nc.vector.tensor_tensor(
                    out=y_t[:W, :], in0=y_t[:W, :], in1=tb_t[:W, :],
                    op=mybir.AluOpType.add,
                )
                nc.sync.dma_start(
                    out_ext[w * W:(w + 1) * W, :],
                    omw_t[:W, :] if int(os.environ.get("DBG_OM", "0")) else y_t[:W, :],
                )
    nc.compile()
    return nc


def kernel(X, E, Wl, Wr, att, bias, gn_weight, gn_bias, gn_mean_scale, **kw):
    global LAST_RESULTS
    X = np.asarray(X, np.float32)
    E = np.asarray(E)
    Wl = np.asarray(Wl, np.float32)
    Wr = np.asarray(Wr, np.float32)
    att = np.asarray(att, np.float32)
    bias = np.asarray(bias, np.float32)
    gn_weight = np.asarray(gn_weight, np.float32)
    gn_bias = np.asarray(gn_bias, np.float32)
    gn_mean_scale = np.asarray(gn_mean_scale, np.float32)

    loop = np.arange(N, dtype=np.int64)
    src = np.concatenate([np.asarray(E[0], np.int64), loop])
    dst = np.concatenate([np.asarray(E[1], np.int64), loop])
    cl, ch, per_core = _prep_edges(src, dst)

    attb = np.tile(att.reshape(1, F), (128, 1)).astype(np.float32)
    diota = np.tile(np.arange(W, dtype=np.float32), (128, 1))
    gnp = np.stack([gn_weight, gn_bias, gn_mean_scale, bias], axis=1).astype(np.float32)

    icols = per_core[0]["IDX"].shape[1]
    rmcols = per_core[0]["RM"].shape[1]
    nc = _build(cl, ch, icols, rmcols)

    in_maps = []
    for c in range(NCORES):
        in_maps.append(
            dict(
                XS=np.ascontiguousarray(X[c * NSH:(c + 1) * NSH]),
                WL=Wl, WR=Wr, ATTB=attb, GNP=gnp, DIOTA=diota,
                IDX=per_core[c]["IDX"], RM=per_core[c]["RM"],
            )
        )
    trace = bool(kw.get("trace"))
    res = run_bass_kernel_spmd(
        nc, in_maps, core_ids=list(range(NCORES)), trace=trace
    )
    LAST_RESULTS = res
    return np.concatenate([res.results[c]["OUT"] for c in range(NCORES)], axis=0)


# revision 11
# speedup vs baseline: 1.0750x; 1.0750x over previous
"""GATv2Conv (heads=4, concat=False, self-loops) + GraphNorm on 8 TRN2 NeuronCores.

v2: bf16 datapath, B-side (xr[dst]) realized via contiguous window DMA +
PE one-hot pick-matmul instead of per-edge gather; denominators fused into
the scatter PSUM tile.

Sharding: destination nodes split into 8 contiguous ranges (6250 each).
Each core computes xl/xr for its own range (bf16), all-gathers xl, then
processes its incoming edges in 125-dst windows: per 128-edge chunk a
dma_gather fetches xl[src] rows, DVE computes GATv2 logits, ACT the
unnormalized softmax weights, and PE one-hot matmuls segment-reduce both
softmax denominators and weighted feature sums into PSUM. A final
AllReduce + affine pass applies GraphNorm.
"""
import os
import sys

sys.path.insert(0, "/opt/trn_rl_repo")

import numpy as np
from concourse import bacc, mybir, tile
from concourse.bass_utils import run_bass_kernel_spmd
from concourse.masks import make_identity

N = 50000
NCORES = 8
NSH = N // NCORES          # 6250 dst nodes per core
IN_F = 256
H = 4
C = 64
F = H * C                  # 256
W = 125                    # dst window size
NW = NSH // W              # 50 windows per core
HALF = N // 2              # gather table half size (int16 index limit)
NEG = 0.2
EPS = 1e-5
MASKVAL = -100.0           # logit bias for padding edges -> exp == 0
CHUNK = 128
GRP = 4                    # chunks fused per DVE op group

f32 = mybir.dt.float32
bf16 = mybir.dt.bfloat16
i16 = mybir.dt.int16

LAST_RESULTS = None


def _pack_idx(idx: np.ndarray) -> np.ndarray:
    """[n] int -> [128, n//16] int16 gather-index layout (16-partition wrap,
    replicated for the 8 Q7 cores)."""
    n = idx.shape[0]
    pk = np.zeros((16, n // 16), np.int16)
    pk[np.arange(n) % 16, np.arange(n) // 16] = idx.astype(np.int16)
    return np.tile(pk, (8, 1))


def _prep_edges(src: np.ndarray, dst: np.ndarray):
    """Partition/sort/pad edges. Returns (cl, ch, per_core) where cl/ch are
    per-window chunk counts (shared across cores) and per_core is a list of
    dicts with IDX (int16 [128, icols]) and RM (f32 [128, rmcols])."""
    src = src.astype(np.int64)
    dst = dst.astype(np.int64)
    core = dst // NSH
    per_core_raw = []
    nlo = np.zeros((NCORES, NW), np.int64)
    nhi = np.zeros((NCORES, NW), np.int64)
    for c in range(NCORES):
        m = core == c
        s = src[m]
        d = dst[m] - c * NSH
        win = d // W
        half = s // HALF
        order = np.lexsort((half, win))
        s, d, win, half = s[order], d[order], win[order], half[order]
        key = win * 2 + half
        cnt = np.bincount(key, minlength=NW * 2).reshape(NW, 2)
        nlo[c] = cnt[:, 0]
        nhi[c] = cnt[:, 1]
        per_core_raw.append((s, d, np.cumsum(cnt.reshape(-1))))
    cl = np.ceil(nlo.max(axis=0) / CHUNK).astype(int)
    ch = np.ceil(nhi.max(axis=0) / CHUNK).astype(int)

    per_core = []
    for c in range(NCORES):
        s, d, cum = per_core_raw[c]
        idx_cols = []
        rm_cols = []
        for w in range(NW):
            beg_lo = cum[w * 2 - 1] if w * 2 > 0 else 0
            end_lo = cum[w * 2]
            end_hi = cum[w * 2 + 1]
            s_lo, d_lo = s[beg_lo:end_lo], d[beg_lo:end_lo]
            s_hi, d_hi = s[end_lo:end_hi], d[end_lo:end_hi]
            nL, nH = cl[w] * CHUNK, ch[w] * CHUNK
            padL = nL - len(s_lo)
            padH = nH - len(s_hi)
            a_lo = np.concatenate([s_lo, np.zeros(padL, np.int64)])
            a_hi = np.concatenate([s_hi - HALF, np.zeros(padH, np.int64)])
            rr = np.concatenate(
                [d_lo - w * W, np.zeros(padL, np.int64),
                 d_hi - w * W, np.zeros(padH, np.int64)]
            ).astype(np.float32)
            mm = np.concatenate(
                [np.zeros(len(s_lo), np.float32), np.full(padL, MASKVAL, np.float32),
                 np.zeros(len(s_hi), np.float32), np.full(padH, MASKVAL, np.float32)]
            )
            cols = []
            if nL:
                cols.append(_pack_idx(a_lo))
            if nH:
                cols.append(_pack_idx(a_hi))
            idx_cols.append(np.concatenate(cols, axis=1))
            cpw = cl[w] + ch[w]
            rm = np.empty((128, 2 * cpw), np.float32)
            rm[:, :cpw] = rr.reshape(cpw, CHUNK).T
            rm[:, cpw:] = mm.reshape(cpw, CHUNK).T
            rm_cols.append(rm)
        per_core.append(
            dict(
                IDX=np.concatenate(idx_cols, axis=1),
                RM=np.concatenate(rm_cols, axis=1),
            )
        )
    return cl, ch, per_core


def _build(cl, ch, icols, rmcols):
    """Build the 8-core SPMD graph. cl/ch: per-window chunk counts."""
    nc = bacc.Bacc(None, target_bir_lowering=False, debug=False, num_devices=NCORES)

    xs_in = nc.declare_dram_parameter("XS", [NSH, IN_F], f32, isOutput=False)
    wl_in = nc.declare_dram_parameter("WL", [IN_F, F], f32, isOutput=False)
    wr_in = nc.declare_dram_parameter("WR", [IN_F, F], f32, isOutput=False)
    attb_in = nc.declare_dram_parameter("ATTB", [128, F], f32, isOutput=False)
    gnp_in = nc.declare_dram_parameter("GNP", [C, 4], f32, isOutput=False)
    diota_in = nc.declare_dram_parameter("DIOTA", [128, W], f32, isOutput=False)
    idx_in = nc.declare_dram_parameter("IDX", [128, icols], i16, isOutput=False)
    rm_in = nc.declare_dram_parameter("RM", [128, rmcols], f32, isOutput=False)
    out_ext = nc.declare_dram_parameter("OUT", [NSH, C], f32, isOutput=True)

    xl_sh = nc.dram_tensor("xl_sh", [NSH, F], bf16)
    xr_d = nc.dram_tensor("xr_d", [NSH, F], bf16)
    xl_full = nc.dram_tensor("xl_full", [N, F], bf16, addr_space="Shared")
    om_d = nc.dram_tensor("om_d", [NSH, C], f32)
    stats_l = nc.dram_tensor("stats_l", [C, 2], f32)
    stats_g = nc.dram_tensor("stats_g", [C, 2], f32, addr_space="Shared")

    cpw = [int(cl[w] + ch[w]) for w in range(NW)]
    cpw_max = max(cpw)
    ioff = np.concatenate([[0], np.cumsum([cpw[w] * 8 for w in range(NW)])])
    roff = np.concatenate([[0], np.cumsum([2 * cpw[w] for w in range(NW)])])

    dbg_om = bool(int(os.environ.get("DBG_OM", "0")))

    with tile.TileContext(nc) as tc:
        with (
            tc.tile_pool(name="const", bufs=1) as cp,
            tc.tile_pool(name="sb", bufs=2) as sb,
            tc.tile_pool(name="acc", bufs=1, space="PSUM") as accp,
        ):
            ident = cp.tile([128, 128], f32)
            make_identity(nc, ident[:])
            identb = cp.tile([128, 128], bf16)
            nc.vector.tensor_copy(out=identb[:], in_=ident[:])
            wl_t = cp.tile([128, 2, F], bf16)
            wr_t = cp.tile([128, 2, F], bf16)
            wl_f = cp.tile([128, 2, F], f32)
            wr_f = cp.tile([128, 2, F], f32)
            nc.sync.dma_start(wl_f[:], wl_in.ap().rearrange("(s k) n -> k s n", k=128))
            nc.sync.dma_start(wr_f[:], wr_in.ap().rearrange("(s k) n -> k s n", k=128))
            nc.vector.tensor_copy(out=wl_t[:], in_=wl_f[:])
            nc.vector.tensor_copy(out=wr_t[:], in_=wr_f[:])
            attb_t = cp.tile([128, F], bf16)
            attb_f = cp.tile([128, F], f32)
            nc.sync.dma_start(attb_f[:], attb_in[:, :])
            nc.vector.tensor_copy(out=attb_t[:], in_=attb_f[:])
            gnp_t = cp.tile([128, 4], f32)
            nc.sync.dma_start(gnp_t[:C, :], gnp_in[:, :])
            diota_t = cp.tile([128, W], f32)
            nc.sync.dma_start(diota_t[:], diota_in[:, :])
            ones_n = cp.tile([128, 1], f32)
            nc.vector.memset(ones_n[:], 1.0)
            ones_r = cp.tile([128, W], f32)
            nc.vector.memset(ones_r[0:1, :], 1.0)
            biasb_t = cp.tile([128, C], f32)
            sb_t = cp.tile([128, C], f32)
            tb_t = cp.tile([128, C], f32)

            # biasB: [W, C] replicated GAT bias (gnp col 3)
            with tc.tile_pool(name="psi", bufs=1, space="PSUM") as psi:
                brow_ps = psi.tile([128, C], f32, space="PSUM", tag="brow")
                nc.tensor.transpose(brow_ps[0:1, :], gnp_t[:C, 3:4], ident[:C, :C])
                brow_t = cp.tile([128, C], f32)
                nc.vector.tensor_copy(out=brow_t[0:1, :], in_=brow_ps[0:1, :])
                biasb_ps = psi.tile([128, C], f32, space="PSUM", tag="bb")
                nc.tensor.matmul(
                    biasb_ps[:W, :], lhsT=ones_r[0:1, :], rhs=brow_t[0:1, :],
                    start=True, stop=True,
                )
                nc.vector.tensor_copy(out=biasb_t[:W, :], in_=biasb_ps[:W, :])

            # ---- P0: xl = X @ Wl, xr = X @ Wr for own shard (bf16) ----
            ntile = (NSH + 127) // 128
            with tc.tile_pool(name="ps0", bufs=2, space="PSUM") as ps0:
                for i in range(ntile):
                    rows = min(128, NSH - i * 128)
                    x_t = sb.tile([128, IN_F], f32, tag="p0x")
                    nc.sync.dma_start(x_t[:rows, :], xs_in[i * 128:i * 128 + rows, :])
                    xb_t = sb.tile([128, IN_F], bf16, tag="p0xb")
                    nc.vector.tensor_copy(out=xb_t[:rows, :], in_=x_t[:rows, :])
                    xt_sb = sb.tile([128, 2, 128], bf16, tag="p0xt")
                    for hh in range(2):
                        xt_ps = ps0.tile([128, 128], bf16, space="PSUM", tag="p0tp")
                        nc.tensor.transpose(
                            xt_ps[:, :rows], xb_t[:rows, hh * 128:(hh + 1) * 128],
                            identb[:rows, :rows],
                        )
                        nc.vector.tensor_copy(out=xt_sb[:, hh, :rows], in_=xt_ps[:, :rows])
                    for w_t, dram in ((wl_t, xl_sh), (wr_t, xr_d)):
                        mm_ps = ps0.tile([128, F], f32, space="PSUM", tag="p0mm")
                        for hh in range(2):
                            nc.tensor.matmul(
                                mm_ps[:rows, :], lhsT=xt_sb[:, hh, :rows],
                                rhs=w_t[:, hh, :], start=(hh == 0), stop=(hh == 1),
                            )
                        o_t = sb.tile([128, F], bf16, tag="p0o")
                        nc.vector.tensor_copy(out=o_t[:rows, :], in_=mm_ps[:rows, :])
                        nc.sync.dma_start(dram[i * 128:i * 128 + rows, :], o_t[:rows, :])

            # ---- P1: all-gather xl ----
            nc.gpsimd.collective_compute(
                "AllGather", mybir.AluOpType.bypass,
                replica_groups=[list(range(NCORES))],
                ins=[xl_sh.ap().opt()], outs=[xl_full.ap().opt()],
            )

            # ---- P2: windowed edge processing ----
            sum_ps = accp.tile([128, 1], f32, space="PSUM", tag="accsum")
            ssq_ps = accp.tile([128, 1], f32, space="PSUM", tag="accssq")
            with tc.tile_pool(name="ps2", bufs=2, space="PSUM") as ps2:
                for w in range(NW):
                    nL, nH, nT = int(cl[w]), int(ch[w]), cpw[w]
                    ix_t = sb.tile([128, cpw_max * 8], i16, tag="ix")
                    nc.sync.dma_start(
                        ix_t[:, :nT * 8], idx_in[:, int(ioff[w]):int(ioff[w + 1])]
                    )
                    rm_t = sb.tile([128, 2 * cpw_max], f32, tag="rm")
                    nc.sync.dma_start(
                        rm_t[:, :2 * nT], rm_in[:, int(roff[w]):int(roff[w + 1])]
                    )
                    a_t = sb.tile([128, cpw_max, F], bf16, tag="a")
                    if nL:
                        nc.gpsimd.dma_gather(
                            a_t[:, :nL, :], xl_full[0:HALF, :], ix_t[:, :nL * 8],
                            nL * CHUNK, nL * CHUNK, F,
                            single_packet=(nL * CHUNK <= 1024),
                        )
                    if nH:
                        nc.gpsimd.dma_gather(
                            a_t[:, nL:nT, :], xl_full[HALF:N, :],
                            ix_t[:, nL * 8:nT * 8], nH * CHUNK, nH * CHUNK, F,
                            single_packet=(nH * CHUNK <= 1024),
                        )
                    xrw_t = sb.tile([128, F], bf16, tag="xrw")
                    nc.sync.dma_start(xrw_t[:W, :], xr_d[w * W:(w + 1) * W, :])
                    fused_ps = ps2.tile([W, F + H], f32, space="PSUM", tag="outp")
                    den_ps = ps2.tile([W, H], f32, space="PSUM", tag="den")
                    p_buf = sb.tile([128, cpw_max, H], bf16, tag="p")
                    for k0 in range(0, nT, GRP):
                        g = min(GRP, nT - k0)
                        m014 = sb.tile([128, GRP, W], bf16, tag="m014")
                        nc.vector.tensor_tensor(
                            out=m014[:, :g, :],
                            in0=diota_t[:].rearrange("p (o w) -> p o w", o=1).to_broadcast([128, g, W]),
                            in1=rm_t[:, k0:k0 + g].rearrange("p (k o) -> p k o", o=1).to_broadcast([128, g, W]),
                            op=mybir.AluOpType.is_equal,
                        )
                        s4 = sb.tile([128, GRP, F], bf16, tag="s4")
                        for j in range(g):
                            m01t_ps = ps2.tile([W, 128], bf16, space="PSUM", tag="m01t", bufs=1)
                            nc.tensor.transpose(m01t_ps[:], m014[:, j, :], identb[:])
                            m01t_t = sb.tile([128, 128], bf16, tag="m01tt")
                            nc.vector.tensor_copy(out=m01t_t[:W, :], in_=m01t_ps[:])
                            b_ps = ps2.tile([128, F], f32, space="PSUM", tag="bps", bufs=1)
                            nc.tensor.matmul(
                                b_ps[:], lhsT=m01t_t[:W, :], rhs=xrw_t[:W, :],
                                start=True, stop=True,
                            )
                            nc.vector.tensor_tensor(
                                out=s4[:, j, :], in0=a_t[:, k0 + j, :], in1=b_ps[:],
                                op=mybir.AluOpType.add,
                            )
                        l4 = sb.tile([128, GRP, F], bf16, tag="l4")
                        nc.vector.scalar_tensor_tensor(
                            out=l4[:, :g, :], in0=s4[:, :g, :], scalar=NEG,
                            in1=s4[:, :g, :], op0=mybir.AluOpType.mult,
                            op1=mybir.AluOpType.max,
                        )
                        t4 = sb.tile([128, GRP, F], bf16, tag="t4")
                        nc.vector.tensor_tensor(
                            out=t4[:, :g, :], in0=l4[:, :g, :],
                            in1=attb_t[:].rearrange("p (o f) -> p o f", o=1).to_broadcast([128, g, F]),
                            op=mybir.AluOpType.mult,
                        )
                        lg4 = sb.tile([128, GRP, H], f32, tag="lg4")
                        nc.vector.reduce_sum(
                            out=lg4[:, :g, :],
                            in_=t4[:, :g, :].rearrange("p k (h c) -> p k h c", h=H),
                            axis=mybir.AxisListType.X,
                        )
                        for j in range(g):
                            nc.scalar.activation(
                                p_buf[:, k0 + j, :], lg4[:, j, :],
                                mybir.ActivationFunctionType.Exp,
                                bias=rm_t[:, nT + k0 + j:nT + k0 + j + 1], scale=1.0,
                            )
                        ap4 = sb.tile([128, GRP, H, C], bf16, tag="ap4")
                        nc.vector.tensor_tensor(
                            out=ap4[:, :g, :, :],
                            in0=a_t[:, k0:k0 + g, :].rearrange("p k (h c) -> p k h c", h=H),
                            in1=p_buf[:, k0:k0 + g, :].rearrange("p k (h o) -> p k h o", o=1).to_broadcast([128, g, H, C]),
                            op=mybir.AluOpType.mult,
                        )
                        for j in range(g):
                            k = k0 + j
                            nc.tensor.matmul(
                                den_ps[:], lhsT=m014[:, j, :],
                                rhs=p_buf[:, k, :],
                                start=(k == 0), stop=(k == nT - 1),
                            )
                            nc.tensor.matmul(
                                fused_ps[:, 0:F], lhsT=m014[:, j, :],
                                rhs=ap4[:, j, :, :],
                                start=(k == 0), stop=(k == nT - 1),
                            )
                    # window epilogue
                    rd_t = sb.tile([128, H], f32, tag="rd")
                    nc.vector.reciprocal(rd_t[:W, :], den_ps[:])
                    oh_t = sb.tile([128, H, C], f32, tag="oh")
                    for h in range(H):
                        nc.vector.tensor_scalar(
                            out=oh_t[:W, h, :], in0=fused_ps[:, h * C:(h + 1) * C],
                            scalar1=rd_t[:W, h:h + 1], scalar2=0.25,
                            op0=mybir.AluOpType.mult, op1=mybir.AluOpType.mult,
                        )
                    o01 = sb.tile([128, C], f32, tag="o01")
                    nc.vector.tensor_tensor(
                        out=o01[:W, :], in0=oh_t[:W, 0, :], in1=oh_t[:W, 1, :],
                        op=mybir.AluOpType.add,
                    )
                    o23 = sb.tile([128, C], f32, tag="o23")
                    nc.vector.tensor_tensor(
                        out=o23[:W, :], in0=oh_t[:W, 2, :], in1=oh_t[:W, 3, :],
                        op=mybir.AluOpType.add,
                    )
                    o0123 = sb.tile([128, C], f32, tag="o0123")
                    nc.vector.tensor_tensor(
                        out=o0123[:W, :], in0=o01[:W, :], in1=o23[:W, :],
                        op=mybir.AluOpType.add,
                    )
                    om_t = sb.tile([128, C], f32, tag="om")
                    nc.vector.tensor_tensor(
                        out=om_t[:W, :], in0=o0123[:W, :], in1=biasb_t[:W, :],
                        op=mybir.AluOpType.add,
                    )
                    sq_t = sb.tile([128, C], f32, tag="sq")
                    nc.scalar.square(sq_t[:W, :], om_t[:W, :])
                    nc.tensor.matmul(
                        sum_ps[:C, :], lhsT=om_t[:W, :], rhs=ones_n[:W, :],
                        start=(w == 0), stop=(w == NW - 1),
                    )
                    nc.tensor.matmul(
                        ssq_ps[:C, :], lhsT=sq_t[:W, :], rhs=ones_n[:W, :],
                        start=(w == 0), stop=(w == NW - 1),
                    )
                    nc.sync.dma_start(om_d[w * W:(w + 1) * W, :], om_t[:W, :])

            # ---- P3: GraphNorm statistics ----
            st_t = sb.tile([128, 2], f32, tag="st")
            nc.vector.tensor_copy(out=st_t[:C, 0:1], in_=sum_ps[:C, :])
            nc.vector.tensor_copy(out=st_t[:C, 1:2], in_=ssq_ps[:C, :])
            nc.sync.dma_start(stats_l[:, :], st_t[:C, :])
            nc.gpsimd.collective_compute(
                "AllReduce", mybir.AluOpType.add,
                replica_groups=[list(range(NCORES))],
                ins=[stats_l.ap().opt()], outs=[stats_g.ap().opt()],
            )
            sg_t = sb.tile([128, 2], f32, tag="sg")
            nc.sync.dma_start(sg_t[:C, :], stats_g[:, :])
            mu_t = sb.tile([128, 1], f32, tag="mu")
            nc.vector.tensor_scalar_mul(mu_t[:C, :], sg_t[:C, 0:1], 1.0 / N)
            msq_t = sb.tile([128, 1], f32, tag="msq")
            nc.vector.tensor_scalar_mul(msq_t[:C, :], sg_t[:C, 1:2], 1.0 / N)
            amu_t = sb.tile([128, 1], f32, tag="amu")
            nc.vector.tensor_tensor(
                out=amu_t[:C, :], in0=gnp_t[:C, 2:3], in1=mu_t[:C, :],
                op=mybir.AluOpType.mult,
            )
            am2_t = sb.tile([128, 1], f32, tag="am2")
            nc.vector.scalar_tensor_tensor(
                out=am2_t[:C, :], in0=mu_t[:C, :], scalar=-2.0, in1=amu_t[:C, :],
                op0=mybir.AluOpType.mult, op1=mybir.AluOpType.add,
            )
            var_t = sb.tile([128, 1], f32, tag="var")
            nc.vector.tensor_tensor(
                out=var_t[:C, :], in0=amu_t[:C, :], in1=am2_t[:C, :],
                op=mybir.AluOpType.mult,
            )
            nc.vector.tensor_tensor(
                out=var_t[:C, :], in0=var_t[:C, :], in1=msq_t[:C, :],
                op=mybir.AluOpType.add,
            )
            nc.vector.tensor_scalar_add(var_t[:C, :], var_t[:C, :], EPS)
            sd_t = sb.tile([128, 1], f32, tag="sd")
            nc.scalar.sqrt(sd_t[:C, :], var_t[:C, :])
            inv_t = sb.tile([128, 1], f32, tag="inv")
            nc.vector.reciprocal(inv_t[:C, :], sd_t[:C, :])
            st2_t = sb.tile([128, 2], f32, tag="st2")
            nc.vector.tensor_tensor(
                out=st2_t[:C, 0:1], in0=gnp_t[:C, 0:1], in1=inv_t[:C, :],
                op=mybir.AluOpType.mult,
            )
            u_t = sb.tile([128, 1], f32, tag="u")
            nc.vector.tensor_tensor(
                out=u_t[:C, :], in0=st2_t[:C, 0:1], in1=amu_t[:C, :],
                op=mybir.AluOpType.mult,
            )
            nc.vector.tensor_tensor(
                out=st2_t[:C, 1:2], in0=gnp_t[:C, 1:2], in1=u_t[:C, :],
                op=mybir.AluOpType.subtract,
            )
            with tc.tile_pool(name="ps3", bufs=1, space="PSUM") as ps3:
                srow_ps = ps3.tile([128, C], f32, space="PSUM", tag="srow")
                nc.tensor.transpose(srow_ps[0:1, :], st2_t[:C, 0:1], ident[:C, :C])
                trow_ps = ps3.tile([128, C], f32, space="PSUM", tag="trow")
                nc.tensor.transpose(trow_ps[0:1, :], st2_t[:C, 1:2], ident[:C, :C])
                srow_t = sb.tile([128, C], f32, tag="srow")
                nc.vector.tensor_copy(out=srow_t[0:1, :], in_=srow_ps[0:1, :])
                trow_t = sb.tile([128, C], f32, tag="trow")
                nc.vector.tensor_copy(out=trow_t[0:1, :], in_=trow_ps[0:1, :])
                sb_ps = ps3.tile([128, C], f32, space="PSUM", tag="sbp")
                nc.tensor.matmul(
                    sb_ps[:W, :], lhsT=ones_r[0:1, :], rhs=srow_t[0:1, :],
                    start=True, stop=True,
                )
                nc.vector.tensor_copy(out=sb_t[:W, :], in_=sb_ps[:W, :])
                tb_ps = ps3.tile([128, C], f32, space="PSUM", tag="tbp")
                nc.tensor.matmul(
                    tb_ps[:W, :], lhsT=ones_r[0:1, :], rhs=trow_t[0:1, :],
                    start=True, stop=True,
                )
                nc.vector.tensor_copy(out=tb_t[:W, :], in_=tb_ps[:W, :])

            # ---- P4: apply GraphNorm affine ----
            for w in range(NW):
                omw_t = sb.tile([128, C], f32, tag="omw")
                nc.sync.dma_start(omw_t[:W, :], om_d[w * W:(w + 1) * W, :])
                y_t = sb.tile([128, C], f32, tag="y")
                nc.vector.tensor_tensor(
                    out=y_t[:W, :], in0=omw_t[:W, :], in1=sb_t[:W, :],
                    op=mybir.AluOpType.mult,
                )
                nc.vector.tensor_tensor(
                    out=y_t[:W, :], in0=y_t[:W, :], in1=tb_t[:W, :],
                    op=mybir.AluOpType.add,
                )
                nc.sync.dma_start(
                    out_ext[w * W:(w + 1) * W, :],
                    omw_t[:W, :] if dbg_om else y_t[:W, :],
                )
    nc.compile()
    return nc


def kernel(X, E, Wl, Wr, att, bias, gn_weight, gn_bias, gn_mean_scale, **kw):
    global LAST_RESULTS
    X = np.asarray(X, np.float32)
    E = np.asarray(E)
    Wl = np.asarray(Wl, np.float32)
    Wr = np.asarray(Wr, np.float32)
    att = np.asarray(att, np.float32)
    bias = np.asarray(bias, np.float32)
    gn_weight = np.asarray(gn_weight, np.float32)
    gn_bias = np.asarray(gn_bias, np.float32)
    gn_mean_scale = np.asarray(gn_mean_scale, np.float32)

    loop = np.arange(N, dtype=np.int64)
    src = np.concatenate([np.asarray(E[0], np.int64), loop])
    dst = np.concatenate([np.asarray(E[1], np.int64), loop])
    cl, ch, per_core = _prep_edges(src, dst)

    attb = np.tile(att.reshape(1, F), (128, 1)).astype(np.float32)
    diota = np.tile(np.arange(W, dtype=np.float32), (128, 1))
    gnp = np.stack([gn_weight, gn_bias, gn_mean_scale, bias], axis=1).astype(np.float32)

    icols = per_core[0]["IDX"].shape[1]
    rmcols = per_core[0]["RM"].shape[1]
    nc = _build(cl, ch, icols, rmcols)

    in_maps = []
    for c in range(NCORES):
        in_maps.append(
            dict(
                XS=np.ascontiguousarray(X[c * NSH:(c + 1) * NSH]),
                WL=Wl, WR=Wr, ATTB=attb, GNP=gnp, DIOTA=diota,
                IDX=per_core[c]["IDX"], RM=per_core[c]["RM"],
            )
        )
    trace = bool(kw.get("trace"))
    res = run_bass_kernel_spmd(
        nc, in_maps, core_ids=list(range(NCORES)), trace=trace
    )
    LAST_RESULTS = res
    return np.concatenate([res.results[c]["OUT"] for c in range(NCORES)], axis=0)


# revision 13
# speedup vs baseline: 1.3919x; 1.2949x over previous
"""GATv2Conv (heads=4, concat=False, self-loops) + GraphNorm on 8 TRN2 NeuronCores.

v3 design notes:
- Edges sharded by destination range (6250 dsts/core), processed in 125-dst
  windows, 128-edge chunks, sorted by (window, src-half).
- Gather table rows are [xl (256) | 0.2*att.xl (4) | pad] bf16 (768B rows so
  dma_gather's 256B-granule constraint holds); xr table rows are
  [xr (256) | 0.2*att.xr (4)].
- Per chunk, PE computes s = xl[src]+xr[dst] and qsum = 0.2*(att.xl[src] +
  att.xr[dst]) in one PSUM tile: a one-hot pick matmul (host-precomputed
  M01T) plus an identity accumulate of the gathered rows.
- leaky_relu(s) = 0.2*s + 0.8*relu(s), so logits = qsum + att08.relu(s)
  where att08 = 0.8*att: ACT computes relu, DVE one multiply by a
  4x-replicated att08 and one reduce, one fused add, ACT exponentiates with
  the padding mask as per-partition bias.
- One matmul per chunk scatters [a*p || p] through M01 into a [125, 260]
  PSUM tile: softmax numerators and denominators in a single accumulation
  group (two interleaved start/stop groups in one PSUM tile corrupt
  accumulation - found the hard way).
- GraphNorm via per-core column sums + AllReduce, then a final affine pass.
"""
import os
import sys

sys.path.insert(0, "/opt/trn_rl_repo")

import ml_dtypes
import numpy as np
from concourse import bacc, mybir, tile
from concourse.bass_utils import run_bass_kernel_spmd
from concourse.masks import make_identity

N = 50000
NCORES = 8
NSH = N // NCORES          # 6250 dst nodes per core
IN_F = 256
H = 4
C = 64
F = H * C                  # 256
FQ = F + H                 # 260: features + q columns
FG = 384                   # gather row width (768B, multiple of 256B)
W = 125                    # dst window size
NW = NSH // W              # 50 windows per core
HALF = N // 2              # gather table half size (int16 index limit)
NEG = 0.2
EPS = 1e-5
MASKVAL = -100.0           # logit bias for padding edges -> exp == 0
CHUNK = 128
GRP = 4                    # chunks fused per DVE op group

f32 = mybir.dt.float32
bf16 = mybir.dt.bfloat16
i16 = mybir.dt.int16

LAST_RESULTS = None


def _pack_idx(idx: np.ndarray) -> np.ndarray:
    """[n] int -> [128, n//16] int16 gather-index layout (16-partition wrap,
    replicated for the 8 Q7 cores)."""
    n = idx.shape[0]
    pk = np.zeros((16, n // 16), np.int16)
    pk[np.arange(n) % 16, np.arange(n) // 16] = idx.astype(np.int16)
    return np.tile(pk, (8, 1))


def _prep_edges(src: np.ndarray, dst: np.ndarray):
    """Partition/sort/pad edges. Returns (cl, ch, per_core); per_core dicts
    hold IDX (int16), MK (f32 mask), M01/M01T (bf16 one-hot blocks)."""
    src = src.astype(np.int64)
    dst = dst.astype(np.int64)
    core = dst // NSH
    per_core_raw = []
    nlo = np.zeros((NCORES, NW), np.int64)
    nhi = np.zeros((NCORES, NW), np.int64)
    for c in range(NCORES):
        m = core == c
        s = src[m]
        d = dst[m] - c * NSH
        win = d // W
        half = s // HALF
        order = np.lexsort((half, win))
        s, d = s[order], d[order]
        key = (d // W) * 2 + (s // HALF)
        cnt = np.bincount(key, minlength=NW * 2).reshape(NW, 2)
        nlo[c] = cnt[:, 0]
        nhi[c] = cnt[:, 1]
        per_core_raw.append((s, d, np.cumsum(cnt.reshape(-1))))
    cl = np.ceil(nlo.max(axis=0) / CHUNK).astype(int)
    ch = np.ceil(nhi.max(axis=0) / CHUNK).astype(int)

    dcols = np.arange(W, dtype=np.int64)
    per_core = []
    for c in range(NCORES):
        s, d, cum = per_core_raw[c]
        idx_cols, mk_cols, m01_cols, m01t_cols = [], [], [], []
        for w in range(NW):
            beg_lo = cum[w * 2 - 1] if w * 2 > 0 else 0
            end_lo = cum[w * 2]
            end_hi = cum[w * 2 + 1]
            s_lo, d_lo = s[beg_lo:end_lo], d[beg_lo:end_lo]
            s_hi, d_hi = s[end_lo:end_hi], d[end_lo:end_hi]
            nL, nH = cl[w] * CHUNK, ch[w] * CHUNK
            padL = nL - len(s_lo)
            padH = nH - len(s_hi)
            a_lo = np.concatenate([s_lo, np.zeros(padL, np.int64)])
            a_hi = np.concatenate([s_hi - HALF, np.zeros(padH, np.int64)])
            rr = np.concatenate(
                [d_lo - w * W, np.zeros(padL, np.int64),
                 d_hi - w * W, np.zeros(padH, np.int64)]
            )
            mm = np.concatenate(
                [np.zeros(len(s_lo), np.float32), np.full(padL, MASKVAL, np.float32),
                 np.zeros(len(s_hi), np.float32), np.full(padH, MASKVAL, np.float32)]
            )
            cols = []
            if nL:
                cols.append(_pack_idx(a_lo))
            if nH:
                cols.append(_pack_idx(a_hi))
            idx_cols.append(np.concatenate(cols, axis=1))
            cpw = cl[w] + ch[w]
            mk_cols.append(mm.reshape(cpw, CHUNK).T.copy())
            rt = rr.reshape(cpw, CHUNK)                      # [k, e]
            m01 = (rt.T[:, :, None] == dcols).astype(ml_dtypes.bfloat16)
            m01_cols.append(m01.reshape(128, cpw * W))
            m01t = (dcols[:, None, None] == rt[None, :, :]).astype(ml_dtypes.bfloat16)
            m01t_pad = np.zeros((128, cpw, CHUNK), ml_dtypes.bfloat16)
            m01t_pad[:W] = m01t
            m01t_cols.append(m01t_pad.reshape(128, cpw * CHUNK))
        per_core.append(
            dict(
                IDX=np.concatenate(idx_cols, axis=1),
                MK=np.concatenate(mk_cols, axis=1).astype(np.float32),
                M01=np.concatenate(m01_cols, axis=1),
                M01T=np.concatenate(m01t_cols, axis=1),
            )
        )
    return cl, ch, per_core


def _build(cl, ch, icols, mkcols, mcols, mtcols):
    """Build the 8-core SPMD graph."""
    nc = bacc.Bacc(None, target_bir_lowering=False, debug=False, num_devices=NCORES)

    xs_in = nc.declare_dram_parameter("XS", [NSH, IN_F], f32, isOutput=False)
    wl_in = nc.declare_dram_parameter("WL", [IN_F, F], f32, isOutput=False)
    wr_in = nc.declare_dram_parameter("WR", [IN_F, F], f32, isOutput=False)
    attb_in = nc.declare_dram_parameter("ATTB", [128, F], f32, isOutput=False)
    gnp_in = nc.declare_dram_parameter("GNP", [C, 4], f32, isOutput=False)
    idx_in = nc.declare_dram_parameter("IDX", [128, icols], i16, isOutput=False)
    mk_in = nc.declare_dram_parameter("MK", [128, mkcols], f32, isOutput=False)
    m01_in = nc.declare_dram_parameter("M01", [128, mcols], bf16, isOutput=False)
    m01t_in = nc.declare_dram_parameter("M01T", [128, mtcols], bf16, isOutput=False)
    out_ext = nc.declare_dram_parameter("OUT", [NSH, C], f32, isOutput=True)

    xl_sh = nc.dram_tensor("xl_sh", [NSH, FG], bf16)
    xr_d = nc.dram_tensor("xr_d", [NSH, FQ], bf16)
    xl_full = nc.dram_tensor("xl_full", [N, FG], bf16, addr_space="Shared")
    om_d = nc.dram_tensor("om_d", [NSH, C], f32)
    stats_l = nc.dram_tensor("stats_l", [C, 2], f32)
    stats_g = nc.dram_tensor("stats_g", [C, 2], f32, addr_space="Shared")

    cpw = [int(cl[w] + ch[w]) for w in range(NW)]
    cpw_max = max(cpw)
    csum = np.concatenate([[0], np.cumsum(cpw)])
    ioff = csum * 8
    dbg_om = bool(int(os.environ.get("DBG_OM", "0")))

    with tile.TileContext(nc) as tc:
        with (
            tc.tile_pool(name="const", bufs=1) as cp,
            tc.tile_pool(name="sb", bufs=2) as sb,
            tc.tile_pool(name="acc", bufs=1, space="PSUM") as accp,
        ):
            ident = cp.tile([128, 128], f32)
            make_identity(nc, ident[:])
            identb = cp.tile([128, 128], bf16)
            nc.vector.tensor_copy(out=identb[:], in_=ident[:])
            wl_t = cp.tile([128, 2, F], bf16)
            wr_t = cp.tile([128, 2, F], bf16)
            wl_f = cp.tile([128, 2, F], f32)
            wr_f = cp.tile([128, 2, F], f32)
            nc.sync.dma_start(wl_f[:], wl_in.ap().rearrange("(s k) n -> k s n", k=128))
            nc.sync.dma_start(wr_f[:], wr_in.ap().rearrange("(s k) n -> k s n", k=128))
            nc.vector.tensor_copy(out=wl_t[:], in_=wl_f[:])
            nc.vector.tensor_copy(out=wr_t[:], in_=wr_f[:])
            # att08: 0.8*att replicated 4x along free dim for group ops
            attb_f = cp.tile([128, F], f32)
            nc.sync.dma_start(attb_f[:], attb_in[:, :])
            att08 = cp.tile([128, F], bf16)
            nc.vector.tensor_scalar_mul(att08[:], attb_f[:], 1.0 - NEG)
            att08g = cp.tile([128, GRP, F], bf16)
            for j in range(GRP):
                nc.vector.tensor_copy(out=att08g[:, j, :], in_=att08[:])
            gnp_t = cp.tile([128, 4], f32)
            nc.sync.dma_start(gnp_t[:C, :], gnp_in[:, :])
            ones_n = cp.tile([128, 1], f32)
            nc.vector.memset(ones_n[:], 1.0)
            ones_r = cp.tile([128, W], f32)
            nc.vector.memset(ones_r[0:1, :], 1.0)
            biasb_t = cp.tile([128, C], f32)
            sb_t = cp.tile([128, C], f32)
            tb_t = cp.tile([128, C], f32)

            # biasB: [W, C] replicated GAT bias (gnp col 3)
            with tc.tile_pool(name="psi", bufs=1, space="PSUM") as psi:
                brow_ps = psi.tile([128, C], f32, space="PSUM", tag="brow")
                nc.tensor.transpose(brow_ps[0:1, :], gnp_t[:C, 3:4], ident[:C, :C])
                brow_t = cp.tile([128, C], f32)
                nc.vector.tensor_copy(out=brow_t[0:1, :], in_=brow_ps[0:1, :])
                biasb_ps = psi.tile([128, C], f32, space="PSUM", tag="bb")
                nc.tensor.matmul(
                    biasb_ps[:W, :], lhsT=ones_r[0:1, :], rhs=brow_t[0:1, :],
                    start=True, stop=True,
                )
                nc.vector.tensor_copy(out=biasb_t[:W, :], in_=biasb_ps[:W, :])

            # ---- P0: xl/xr transforms + q columns (bf16) ----
            ntile = (NSH + 127) // 128
            with tc.tile_pool(name="ps0", bufs=2, space="PSUM") as ps0:
                for i in range(ntile):
                    rows = min(128, NSH - i * 128)
                    x_t = sb.tile([128, IN_F], f32, tag="p0x")
                    nc.sync.dma_start(x_t[:rows, :], xs_in[i * 128:i * 128 + rows, :])
                    xb_t = sb.tile([128, IN_F], bf16, tag="p0xb")
                    nc.vector.tensor_copy(out=xb_t[:rows, :], in_=x_t[:rows, :])
                    xt_sb = sb.tile([128, 2, 128], bf16, tag="p0xt")
                    for hh in range(2):
                        xt_ps = ps0.tile([128, 128], bf16, space="PSUM", tag="p0tp")
                        nc.tensor.transpose(
                            xt_ps[:, :rows], xb_t[:rows, hh * 128:(hh + 1) * 128],
                            identb[:rows, :rows],
                        )
                        nc.vector.tensor_copy(out=xt_sb[:, hh, :rows], in_=xt_ps[:, :rows])
                    for w_t, dram, width in ((wl_t, xl_sh, FG), (wr_t, xr_d, FQ)):
                        mm_ps = ps0.tile([128, F], f32, space="PSUM", tag="p0mm")
                        for hh in range(2):
                            nc.tensor.matmul(
                                mm_ps[:rows, :], lhsT=xt_sb[:, hh, :rows],
                                rhs=w_t[:, hh, :], start=(hh == 0), stop=(hh == 1),
                            )
                        o_t = sb.tile([128, FG], bf16, tag="p0o")
                        nc.vector.tensor_copy(out=o_t[:rows, 0:F], in_=mm_ps[:rows, :])
                        # q columns: 0.2 * att . x{l,r}  (per head)
                        for h in range(H):
                            scr_t = sb.tile([128, C], bf16, tag="p0scr")
                            nc.vector.scalar_tensor_tensor(
                                out=scr_t[:rows, :],
                                in0=mm_ps[:rows, h * C:(h + 1) * C], scalar=NEG,
                                in1=attb_f[:rows, h * C:(h + 1) * C],
                                op0=mybir.AluOpType.mult, op1=mybir.AluOpType.mult,
                                accum_out=o_t[:rows, F + h:F + h + 1],
                            )
                        nc.sync.dma_start(
                            dram[i * 128:i * 128 + rows, :],
                            o_t[:rows, 0:width],
                        )

            # ---- P1: all-gather xl ----
            nc.gpsimd.collective_compute(
                "AllGather", mybir.AluOpType.bypass,
                replica_groups=[list(range(NCORES))],
                ins=[xl_sh.ap().opt()], outs=[xl_full.ap().opt()],
            )

            # ---- P2: windowed edge processing ----
            sum_ps = accp.tile([128, 1], f32, space="PSUM", tag="accsum")
            ssq_ps = accp.tile([128, 1], f32, space="PSUM", tag="accssq")
            with tc.tile_pool(name="ps2", bufs=2, space="PSUM") as ps2:
                for w in range(NW):
                    nL, nH, nT = int(cl[w]), int(ch[w]), cpw[w]
                    ix_t = sb.tile([128, cpw_max * 8], i16, tag="ix")
                    nc.sync.dma_start(
                        ix_t[:, :nT * 8], idx_in[:, int(ioff[w]):int(ioff[w + 1])]
                    )
                    mk_t = sb.tile([128, cpw_max], f32, tag="mk")
                    nc.sync.dma_start(
                        mk_t[:, :nT], mk_in[:, int(csum[w]):int(csum[w + 1])]
                    )
                    m01_t = sb.tile([128, cpw_max, W], bf16, tag="m01")
                    nc.sync.dma_start(
                        m01_t[:, :nT, :],
                        m01_in[:, int(csum[w]) * W:int(csum[w + 1]) * W],
                    )
                    m01t_t = sb.tile([128, cpw_max, CHUNK], bf16, tag="m01t")
                    nc.sync.dma_start(
                        m01t_t[:, :nT, :],
                        m01t_in[:, int(csum[w]) * CHUNK:int(csum[w + 1]) * CHUNK],
                    )
                    a_t = sb.tile([128, cpw_max, FG], bf16, tag="a")
                    if nL:
                        nc.gpsimd.dma_gather(
                            a_t[:, :nL, :], xl_full[0:HALF, :], ix_t[:, :nL * 8],
                            nL * CHUNK, nL * CHUNK, FG,
                            single_packet=(nL * CHUNK <= 1024),
                        )
                    if nH:
                        nc.gpsimd.dma_gather(
                            a_t[:, nL:nT, :], xl_full[HALF:N, :],
                            ix_t[:, nL * 8:nT * 8], nH * CHUNK, nH * CHUNK, FG,
                            single_packet=(nH * CHUNK <= 1024),
                        )
                    xrq_t = sb.tile([128, FQ], bf16, tag="xrq")
                    nc.sync.dma_start(xrq_t[:W, :], xr_d[w * W:(w + 1) * W, :])
                    out_ps = ps2.tile([W, FQ], f32, space="PSUM", tag="outp")
                    for k0 in range(0, nT, GRP):
                        g = min(GRP, nT - k0)
                        rs4 = sb.tile([128, GRP, F], bf16, tag="rs4")
                        ap4 = sb.tile([128, GRP, FQ], bf16, tag="ap4")
                        lgr4 = sb.tile([128, GRP, H], f32, tag="lgr4")
                        for j in range(g):
                            k = k0 + j
                            b_ps = ps2.tile([128, FQ], f32, space="PSUM", tag="bps")
                            nc.tensor.matmul(
                                b_ps[:], lhsT=m01t_t[:W, k, :], rhs=xrq_t[:W, :],
                                start=True, stop=False,
                            )
                            nc.tensor.matmul(
                                b_ps[:], lhsT=identb[:], rhs=a_t[:, k, 0:FQ],
                                start=False, stop=True,
                            )
                            # relu(s) -> bf16
                            nc.scalar.activation(
                                rs4[:, j, :], b_ps[:, 0:F],
                                mybir.ActivationFunctionType.Relu,
                            )
                            # stash qsum (b_ps is double-buffered per chunk)
                            nc.scalar.copy(lgr4[:, j, :], b_ps[:, F:FQ])
                        t4 = sb.tile([128, GRP, F], bf16, tag="t4")
                        nc.vector.tensor_tensor(
                            out=t4[:, :g, :], in0=rs4[:, :g, :],
                            in1=att08g[:, :g, :], op=mybir.AluOpType.mult,
                        )
                        red4 = sb.tile([128, GRP, H], f32, tag="red4")
                        nc.vector.reduce_sum(
                            out=red4[:, :g, :],
                            in_=t4[:, :g, :].rearrange("p k (h c) -> p k h c", h=H),
                            axis=mybir.AxisListType.X,
                        )
                        lg4 = sb.tile([128, GRP, H], f32, tag="lg4")
                        nc.vector.tensor_tensor(
                            out=lg4[:, :g, :], in0=red4[:, :g, :],
                            in1=lgr4[:, :g, :], op=mybir.AluOpType.add,
                        )
                        for j in range(g):
                            k = k0 + j
                            nc.scalar.activation(
                                ap4[:, j, F:FQ], lg4[:, j, :],
                                mybir.ActivationFunctionType.Exp,
                                bias=mk_t[:, k:k + 1], scale=1.0,
                            )
                        nc.vector.tensor_tensor(
                            out=ap4[:, :g, 0:F].rearrange("p k (h c) -> p k h c", h=H),
                            in0=a_t[:, k0:k0 + g, 0:F].rearrange("p k (h c) -> p k h c", h=H),
                            in1=ap4[:, :g, F:FQ].rearrange("p k (h o) -> p k h o", o=1).to_broadcast([128, g, H, C]),
                            op=mybir.AluOpType.mult,
                        )
                        for j in range(g):
                            k = k0 + j
                            nc.tensor.matmul(
                                out_ps[:], lhsT=m01_t[:, k, :], rhs=ap4[:, j, :],
                                start=(k == 0), stop=(k == nT - 1),
                            )
                    # window epilogue
                    rd_t = sb.tile([128, H], f32, tag="rd")
                    nc.vector.reciprocal(rd_t[:W, :], out_ps[:, F:FQ])
                    oh_t = sb.tile([128, H, C], f32, tag="oh")
                    for h in range(H):
                        nc.vector.tensor_scalar(
                            out=oh_t[:W, h, :], in0=out_ps[:, h * C:(h + 1) * C],
                            scalar1=rd_t[:W, h:h + 1], scalar2=0.25,
                            op0=mybir.AluOpType.mult, op1=mybir.AluOpType.mult,
                        )
                    o01 = sb.tile([128, C], f32, tag="o01")
                    nc.vector.tensor_tensor(
                        out=o01[:W, :], in0=oh_t[:W, 0, :], in1=oh_t[:W, 1, :],
                        op=mybir.AluOpType.add,
                    )
                    o23 = sb.tile([128, C], f32, tag="o23")
                    nc.vector.tensor_tensor(
                        out=o23[:W, :], in0=oh_t[:W, 2, :], in1=oh_t[:W, 3, :],
                        op=mybir.AluOpType.add,
                    )
                    o0123 = sb.tile([128, C], f32, tag="o0123")
                    nc.vector.tensor_tensor(
                        out=o0123[:W, :], in0=o01[:W, :], in1=o23[:W, :],
                        op=mybir.AluOpType.add,
                    )
                    om_t = sb.tile([128, C], f32, tag="om")
                    nc.vector.tensor_tensor(
                        out=om_t[:W, :], in0=o0123[:W, :], in1=biasb_t[:W, :],
                        op=mybir.AluOpType.add,
                    )
                    sq_t = sb.tile([128, C], f32, tag="sq")
                    nc.scalar.square(sq_t[:W, :], om_t[:W, :])
                    nc.tensor.matmul(
                        sum_ps[:C, :], lhsT=om_t[:W, :], rhs=ones_n[:W, :],
                        start=(w == 0), stop=(w == NW - 1),
                    )
                    nc.tensor.matmul(
                        ssq_ps[:C, :], lhsT=sq_t[:W, :], rhs=ones_n[:W, :],
                        start=(w == 0), stop=(w == NW - 1),
                    )
                    nc.sync.dma_start(om_d[w * W:(w + 1) * W, :], om_t[:W, :])

            # ---- P3: GraphNorm statistics ----
            st_t = sb.tile([128, 2], f32, tag="st")
            nc.vector.tensor_copy(out=st_t[:C, 0:1], in_=sum_ps[:C, :])
            nc.vector.tensor_copy(out=st_t[:C, 1:2], in_=ssq_ps[:C, :])
            nc.sync.dma_start(stats_l[:, :], st_t[:C, :])
            nc.gpsimd.collective_compute(
                "AllReduce", mybir.AluOpType.add,
                replica_groups=[list(range(NCORES))],
                ins=[stats_l.ap().opt()], outs=[stats_g.ap().opt()],
            )
            sg_t = sb.tile([128, 2], f32, tag="sg")
            nc.sync.dma_start(sg_t[:C, :], stats_g[:, :])
            mu_t = sb.tile([128, 1], f32, tag="mu")
            nc.vector.tensor_scalar_mul(mu_t[:C, :], sg_t[:C, 0:1], 1.0 / N)
            msq_t = sb.tile([128, 1], f32, tag="msq")
            nc.vector.tensor_scalar_mul(msq_t[:C, :], sg_t[:C, 1:2], 1.0 / N)
            amu_t = sb.tile([128, 1], f32, tag="amu")
            nc.vector.tensor_tensor(
                out=amu_t[:C, :], in0=gnp_t[:C, 2:3], in1=mu_t[:C, :],
                op=mybir.AluOpType.mult,
            )
            am2_t = sb.tile([128, 1], f32, tag="am2")
            nc.vector.scalar_tensor_tensor(
                out=am2_t[:C, :], in0=mu_t[:C, :], scalar=-2.0, in1=amu_t[:C, :],
                op0=mybir.AluOpType.mult, op1=mybir.AluOpType.add,
            )
            var_t = sb.tile([128, 1], f32, tag="var")
            nc.vector.tensor_tensor(
                out=var_t[:C, :], in0=amu_t[:C, :], in1=am2_t[:C, :],
                op=mybir.AluOpType.mult,
            )
            nc.vector.tensor_tensor(
                out=var_t[:C, :], in0=var_t[:C, :], in1=msq_t[:C, :],
                op=mybir.AluOpType.add,
            )
            nc.vector.tensor_scalar_add(var_t[:C, :], var_t[:C, :], EPS)
            sd_t = sb.tile([128, 1], f32, tag="sd")
            nc.scalar.sqrt(sd_t[:C, :], var_t[:C, :])
            inv_t = sb.tile([128, 1], f32, tag="inv")
            nc.vector.reciprocal(inv_t[:C, :], sd_t[:C, :])
            st2_t = sb.tile([128, 2], f32, tag="st2")
            nc.vector.tensor_tensor(
                out=st2_t[:C, 0:1], in0=gnp_t[:C, 0:1], in1=inv_t[:C, :],
                op=mybir.AluOpType.mult,
            )
            u_t = sb.tile([128, 1], f32, tag="u")
            nc.vector.tensor_tensor(
                out=u_t[:C, :], in0=st2_t[:C, 0:1], in1=amu_t[:C, :],
                op=mybir.AluOpType.mult,
            )
            nc.vector.tensor_tensor(
                out=st2_t[:C, 1:2], in0=gnp_t[:C, 1:2], in1=u_t[:C, :],
                op=mybir.AluOpType.subtract,
            )
            with tc.tile_pool(name="ps3", bufs=1, space="PSUM") as ps3:
                srow_ps = ps3.tile([128, C], f32, space="PSUM", tag="srow")
                nc.tensor.transpose(srow_ps[0:1, :], st2_t[:C, 0:1], ident[:C, :C])
                trow_ps = ps3.tile([128, C], f32, space="PSUM", tag="trow")
                nc.tensor.transpose(trow_ps[0:1, :], st2_t[:C, 1:2], ident[:C, :C])
                srow_t = sb.tile([128, C], f32, tag="srow")
                nc.vector.tensor_copy(out=srow_t[0:1, :], in_=srow_ps[0:1, :])
                trow_t = sb.tile([128, C], f32, tag="trow")
                nc.vector.tensor_copy(out=trow_t[0:1, :], in_=trow_ps[0:1, :])
                sb_ps = ps3.tile([128, C], f32, space="PSUM", tag="sbp")
                nc.tensor.matmul(
                    sb_ps[:W, :], lhsT=ones_r[0:1, :], rhs=srow_t[0:1, :],
                    start=True, stop=True,
                )
                nc.vector.tensor_copy(out=sb_t[:W, :], in_=sb_ps[:W, :])
                tb_ps = ps3.tile([128, C], f32, space="PSUM", tag="tbp")
                nc.tensor.matmul(
                    tb_ps[:W, :], lhsT=ones_r[0:1, :], rhs=trow_t[0:1, :],
                    start=True, stop=True,
                )
                nc.vector.tensor_copy(out=tb_t[:W, :], in_=tb_ps[:W, :])

            # ---- P4: apply GraphNorm affine ----
            for w in range(NW):
                omw_t = sb.tile([128, C], f32, tag="omw")
                nc.sync.dma_start(omw_t[:W, :], om_d[w * W:(w + 1) * W, :])
                y_t = sb.tile([128, C], f32, tag="y")
                nc.vector.tensor_tensor(
                    out=y_t[:W, :], in0=omw_t[:W, :], in1=sb_t[:W, :],
                    op=mybir.AluOpType.mult,
                )
                nc.vector.tensor_tensor(
                    out=y_t[:W, :], in0=y_t[:W, :], in1=tb_t[:W, :],
                    op=mybir.AluOpType.add,
                )
                nc.sync.dma_start(
                    out_ext[w * W:(w + 1) * W, :],
                    omw_t[:W, :] if dbg_om else y_t[:W, :],
                )
    nc.compile()
    return nc


def kernel(X, E, Wl, Wr, att, bias, gn_weight, gn_bias, gn_mean_scale, **kw):
    global LAST_RESULTS
    X = np.asarray(X, np.float32)
    E = np.asarray(E)
    Wl = np.asarray(Wl, np.float32)
    Wr = np.asarray(Wr, np.float32)
    att = np.asarray(att, np.float32)
    bias = np.asarray(bias, np.float32)
    gn_weight = np.asarray(gn_weight, np.float32)
    gn_bias = np.asarray(gn_bias, np.float32)
    gn_mean_scale = np.asarray(gn_mean_scale, np.float32)

    loop = np.arange(N, dtype=np.int64)
    src = np.concatenate([np.asarray(E[0], np.int64), loop])
    dst = np.concatenate([np.asarray(E[1], np.int64), loop])
    cl, ch, per_core = _prep_edges(src, dst)

    attb = np.tile(att.reshape(1, F), (128, 1)).astype(np.float32)
    gnp = np.stack([gn_weight, gn_bias, gn_mean_scale, bias], axis=1).astype(np.float32)

    p0 = per_core[0]
    nc = _build(cl, ch, p0["IDX"].shape[1], p0["MK"].shape[1],
                p0["M01"].shape[1], p0["M01T"].shape[1])

    in_maps = []
    for c in range(NCORES):
        in_maps.append(
            dict(
                XS=np.ascontiguousarray(X[c * NSH:(c + 1) * NSH]),
                WL=Wl, WR=Wr, ATTB=attb, GNP=gnp,
                IDX=per_core[c]["IDX"], MK=per_core[c]["MK"],
                M01=per_core[c]["M01"], M01T=per_core[c]["M01T"],
            )
        )
    trace = bool(kw.get("trace"))
    res = run_bass_kernel_spmd(
        nc, in_maps, core_ids=list(range(NCORES)), trace=trace
    )
    LAST_RESULTS = res
    return np.concatenate([res.results[c]["OUT"] for c in range(NCORES)], axis=0)


# revision 14
# speedup vs baseline: 1.7568x; 1.2621x over previous
"""GATv2Conv (heads=4, concat=False, self-loops) + GraphNorm on 8 TRN2 NeuronCores.

v4 design notes:
- Edges sharded by destination range (6250 dsts/core), processed in 125-dst
  windows, 128-edge chunks, sorted by (window, src-subshard).
- xl is all-gathered as TWO tables (per-core rows [0:3200) and [3200:6250))
  so edge chunks whose sources live in the first table can start gathering
  before the second all-gather lands. Both tables stay under the int16
  gather-index limit.
- Per chunk, PE computes s = xl[src]+xr[dst] in PSUM via a one-hot pick
  matmul (host-precomputed M01T) plus an identity accumulate of the
  gathered rows; ACT applies leaky-relu directly (Prelu, alpha honored,
  unlike the table-fixed Lrelu); DVE multiplies by a replicated att and
  reduces per head; ACT exponentiates with the padding mask as bias.
- One matmul per chunk scatters [a*p || p] through M01 into a [125, 260]
  PSUM tile: softmax numerators and denominators in one accumulation group
  (two interleaved start/stop groups in one PSUM tile corrupt results).
- om (pre-GraphNorm output) stays SBUF-resident; GraphNorm stats via
  AllReduce, then the affine pass writes the output.
"""
import os
import sys

sys.path.insert(0, "/opt/trn_rl_repo")

import ml_dtypes
import numpy as np
from concourse import bacc, mybir, tile
from concourse.bass_utils import run_bass_kernel_spmd
from concourse.masks import make_identity

N = 50000
NCORES = 8
NSH = N // NCORES          # 6250 dst nodes per core
SPL = 3200                 # sub-shard split: rows [0:SPL) -> table A
NA = NCORES * SPL          # 25600 rows in table A
NB = NCORES * (NSH - SPL)  # 24400 rows in table B
IN_F = 256
H = 4
C = 64
F = H * C                  # 256
FQ = F + H                 # 260: scatter rhs = [a*p || p]
W = 125                    # dst window size
NW = NSH // W              # 50 windows per core
NEG = 0.2
EPS = 1e-5
MASKVAL = -100.0           # logit bias for padding edges -> exp == 0
CHUNK = 128
GRP = 4                    # chunks fused per DVE op group

f32 = mybir.dt.float32
bf16 = mybir.dt.bfloat16
i16 = mybir.dt.int16

LAST_RESULTS = None


def _pack_idx(idx: np.ndarray) -> np.ndarray:
    """[n] int -> [128, n//16] int16 gather-index layout (16-partition wrap,
    replicated for the 8 Q7 cores)."""
    n = idx.shape[0]
    pk = np.zeros((16, n // 16), np.int16)
    pk[np.arange(n) % 16, np.arange(n) // 16] = idx.astype(np.int16)
    return np.tile(pk, (8, 1))


def _prep_edges(src: np.ndarray, dst: np.ndarray):
    """Partition/sort/pad edges. Returns (cl, ch, per_core); per_core dicts
    hold IDX (int16), MK (f32 mask), M01/M01T (bf16 one-hot blocks)."""
    src = src.astype(np.int64)
    dst = dst.astype(np.int64)
    core = dst // NSH
    scid = src // NSH                  # source core
    soff = src % NSH                   # offset within source shard
    is_b = soff >= SPL
    tidx = np.where(is_b, scid * (NSH - SPL) + soff - SPL, scid * SPL + soff)
    per_core_raw = []
    nlo = np.zeros((NCORES, NW), np.int64)
    nhi = np.zeros((NCORES, NW), np.int64)
    for c in range(NCORES):
        m = core == c
        ti = tidx[m]
        hb = is_b[m].astype(np.int64)
        d = dst[m] - c * NSH
        win = d // W
        order = np.lexsort((hb, win))
        ti, d, hb = ti[order], d[order], hb[order]
        key = (d // W) * 2 + hb
        cnt = np.bincount(key, minlength=NW * 2).reshape(NW, 2)
        nlo[c] = cnt[:, 0]
        nhi[c] = cnt[:, 1]
        per_core_raw.append((ti, d, np.cumsum(cnt.reshape(-1))))
    cl = np.ceil(nlo.max(axis=0) / CHUNK).astype(int)
    ch = np.ceil(nhi.max(axis=0) / CHUNK).astype(int)

    dcols = np.arange(W, dtype=np.int64)
    per_core = []
    for c in range(NCORES):
        ti, d, cum = per_core_raw[c]
        idx_cols, mk_cols, m01_cols, m01t_cols = [], [], [], []
        for w in range(NW):
            beg_lo = cum[w * 2 - 1] if w * 2 > 0 else 0
            end_lo = cum[w * 2]
            end_hi = cum[w * 2 + 1]
            t_lo, d_lo = ti[beg_lo:end_lo], d[beg_lo:end_lo]
            t_hi, d_hi = ti[end_lo:end_hi], d[end_lo:end_hi]
            nL, nH = cl[w] * CHUNK, ch[w] * CHUNK
            padL = nL - len(t_lo)
            padH = nH - len(t_hi)
            a_lo = np.concatenate([t_lo, np.zeros(padL, np.int64)])
            a_hi = np.concatenate([t_hi, np.zeros(padH, np.int64)])
            rr = np.concatenate(
                [d_lo - w * W, np.zeros(padL, np.int64),
                 d_hi - w * W, np.zeros(padH, np.int64)]
            )
            mm = np.concatenate(
                [np.zeros(len(t_lo), np.float32), np.full(padL, MASKVAL, np.float32),
                 np.zeros(len(t_hi), np.float32), np.full(padH, MASKVAL, np.float32)]
            )
            cols = []
            if nL:
                cols.append(_pack_idx(a_lo))
            if nH:
                cols.append(_pack_idx(a_hi))
            idx_cols.append(np.concatenate(cols, axis=1))
            cpw = cl[w] + ch[w]
            mk_cols.append(mm.reshape(cpw, CHUNK).T.copy())
            rt = rr.reshape(cpw, CHUNK)                      # [k, e]
            m01 = (rt.T[:, :, None] == dcols).astype(ml_dtypes.bfloat16)
            m01_cols.append(m01.reshape(128, cpw * W))
            m01t = (dcols[:, None, None] == rt[None, :, :]).astype(ml_dtypes.bfloat16)
            m01t_pad = np.zeros((128, cpw, CHUNK), ml_dtypes.bfloat16)
            m01t_pad[:W] = m01t
            m01t_cols.append(m01t_pad.reshape(128, cpw * CHUNK))
        per_core.append(
            dict(
                IDX=np.concatenate(idx_cols, axis=1),
                MK=np.concatenate(mk_cols, axis=1).astype(np.float32),
                M01=np.concatenate(m01_cols, axis=1),
                M01T=np.concatenate(m01t_cols, axis=1),
            )
        )
    return cl, ch, per_core


def _build(cl, ch, icols, mkcols, mcols, mtcols):
    """Build the 8-core SPMD graph."""
    nc = bacc.Bacc(None, target_bir_lowering=False, debug=False, num_devices=NCORES)

    xs_in = nc.declare_dram_parameter("XS", [NSH, IN_F], f32, isOutput=False)
    wl_in = nc.declare_dram_parameter("WL", [IN_F, F], f32, isOutput=False)
    wr_in = nc.declare_dram_parameter("WR", [IN_F, F], f32, isOutput=False)
    attb_in = nc.declare_dram_parameter("ATTB", [128, F], f32, isOutput=False)
    gnp_in = nc.declare_dram_parameter("GNP", [C, 4], f32, isOutput=False)
    idx_in = nc.declare_dram_parameter("IDX", [128, icols], i16, isOutput=False)
    mk_in = nc.declare_dram_parameter("MK", [128, mkcols], f32, isOutput=False)
    m01_in = nc.declare_dram_parameter("M01", [128, mcols], bf16, isOutput=False)
    m01t_in = nc.declare_dram_parameter("M01T", [128, mtcols], bf16, isOutput=False)
    out_ext = nc.declare_dram_parameter("OUT", [NSH, C], f32, isOutput=True)

    xl_sha = nc.dram_tensor("xl_sha", [SPL, F], bf16)
    xl_shb = nc.dram_tensor("xl_shb", [NSH - SPL, F], bf16)
    xr_d = nc.dram_tensor("xr_d", [NSH, F], bf16)
    xl_fa = nc.dram_tensor("xl_fa", [NA, F], bf16, addr_space="Shared")
    xl_fb = nc.dram_tensor("xl_fb", [NB, F], bf16, addr_space="Shared")
    stats_l = nc.dram_tensor("stats_l", [C, 2], f32)
    stats_g = nc.dram_tensor("stats_g", [C, 2], f32, addr_space="Shared")

    cpw = [int(cl[w] + ch[w]) for w in range(NW)]
    cpw_max = max(cpw)
    csum = np.concatenate([[0], np.cumsum(cpw)])
    ioff = csum * 8
    dbg_om = bool(int(os.environ.get("DBG_OM", "0")))

    with tile.TileContext(nc) as tc:
        with (
            tc.tile_pool(name="const", bufs=1) as cp,
            tc.tile_pool(name="sb", bufs=2) as sb,
            tc.tile_pool(name="acc", bufs=1, space="PSUM") as accp,
        ):
            ident = cp.tile([128, 128], f32)
            make_identity(nc, ident[:])
            identb = cp.tile([128, 128], bf16)
            nc.vector.tensor_copy(out=identb[:], in_=ident[:])
            wl_t = cp.tile([128, 2, F], bf16)
            wr_t = cp.tile([128, 2, F], bf16)
            wl_f = cp.tile([128, 2, F], f32)
            wr_f = cp.tile([128, 2, F], f32)
            nc.sync.dma_start(wl_f[:], wl_in.ap().rearrange("(s k) n -> k s n", k=128))
            nc.sync.dma_start(wr_f[:], wr_in.ap().rearrange("(s k) n -> k s n", k=128))
            nc.vector.tensor_copy(out=wl_t[:], in_=wl_f[:])
            nc.vector.tensor_copy(out=wr_t[:], in_=wr_f[:])
            # att replicated 4x along free dim for group ops
            attb_f = cp.tile([128, F], f32)
            nc.sync.dma_start(attb_f[:], attb_in[:, :])
            attg = cp.tile([128, GRP, F], bf16)
            for j in range(GRP):
                nc.vector.tensor_copy(out=attg[:, j, :], in_=attb_f[:])
            gnp_t = cp.tile([128, 4], f32)
            nc.sync.dma_start(gnp_t[:C, :], gnp_in[:, :])
            ones_n = cp.tile([128, 1], f32)
            nc.vector.memset(ones_n[:], 1.0)
            ones_r = cp.tile([128, W], f32)
            nc.vector.memset(ones_r[0:1, :], 1.0)
            biasb_t = cp.tile([128, C], f32)
            sb_t = cp.tile([128, C], f32)
            tb_t = cp.tile([128, C], f32)
            om_all = cp.tile([128, NW, C], f32)

            # biasB: [W, C] replicated GAT bias (gnp col 3)
            with tc.tile_pool(name="psi", bufs=1, space="PSUM") as psi:
                brow_ps = psi.tile([128, C], f32, space="PSUM", tag="brow")
                nc.tensor.transpose(brow_ps[0:1, :], gnp_t[:C, 3:4], ident[:C, :C])
                brow_t = cp.tile([128, C], f32)
                nc.vector.tensor_copy(out=brow_t[0:1, :], in_=brow_ps[0:1, :])
                biasb_ps = psi.tile([128, C], f32, space="PSUM", tag="bb")
                nc.tensor.matmul(
                    biasb_ps[:W, :], lhsT=ones_r[0:1, :], rhs=brow_t[0:1, :],
                    start=True, stop=True,
                )
                nc.vector.tensor_copy(out=biasb_t[:W, :], in_=biasb_ps[:W, :])

            # ---- P0: xl/xr transforms (bf16) ----
            ntile = (NSH + 127) // 128
            with tc.tile_pool(name="ps0", bufs=2, space="PSUM") as ps0:
                for i in range(ntile):
                    rows = min(128, NSH - i * 128)
                    x_t = sb.tile([128, IN_F], f32, tag="p0x")
                    nc.sync.dma_start(x_t[:rows, :], xs_in[i * 128:i * 128 + rows, :])
                    xb_t = sb.tile([128, IN_F], bf16, tag="p0xb")
                    nc.vector.tensor_copy(out=xb_t[:rows, :], in_=x_t[:rows, :])
                    xt_sb = sb.tile([128, 2, 128], bf16, tag="p0xt")
                    for hh in range(2):
                        xt_ps = ps0.tile([128, 128], bf16, space="PSUM", tag="p0tp")
                        nc.tensor.transpose(
                            xt_ps[:, :rows], xb_t[:rows, hh * 128:(hh + 1) * 128],
                            identb[:rows, :rows],
                        )
                        nc.vector.tensor_copy(out=xt_sb[:, hh, :rows], in_=xt_ps[:, :rows])
                    if i * 128 < SPL:
                        xl_dram, xl_row = xl_sha, i * 128
                    else:
                        xl_dram, xl_row = xl_shb, i * 128 - SPL
                    for w_t, dram, row0 in ((wl_t, xl_dram, xl_row), (wr_t, xr_d, i * 128)):
                        mm_ps = ps0.tile([128, F], f32, space="PSUM", tag="p0mm")
                        for hh in range(2):
                            nc.tensor.matmul(
                                mm_ps[:rows, :], lhsT=xt_sb[:, hh, :rows],
                                rhs=w_t[:, hh, :], start=(hh == 0), stop=(hh == 1),
                            )
                        o_t = sb.tile([128, F], bf16, tag="p0o")
                        nc.vector.tensor_copy(out=o_t[:rows, :], in_=mm_ps[:rows, :])
                        nc.sync.dma_start(dram[row0:row0 + rows, :], o_t[:rows, :])

            # ---- P1: all-gather xl (two halves; A lands first) ----
            nc.gpsimd.collective_compute(
                "AllGather", mybir.AluOpType.bypass,
                replica_groups=[list(range(NCORES))],
                ins=[xl_sha.ap().opt()], outs=[xl_fa.ap().opt()],
            )
            nc.gpsimd.collective_compute(
                "AllGather", mybir.AluOpType.bypass,
                replica_groups=[list(range(NCORES))],
                ins=[xl_shb.ap().opt()], outs=[xl_fb.ap().opt()],
            )

            # ---- P2: windowed edge processing ----
            sum_ps = accp.tile([128, 1], f32, space="PSUM", tag="accsum")
            ssq_ps = accp.tile([128, 1], f32, space="PSUM", tag="accssq")
            with tc.tile_pool(name="ps2", bufs=2, space="PSUM") as ps2:
                for w in range(NW):
                    nL, nH, nT = int(cl[w]), int(ch[w]), cpw[w]
                    ix_t = sb.tile([128, cpw_max * 8], i16, tag="ix")
                    nc.sync.dma_start(
                        ix_t[:, :nT * 8], idx_in[:, int(ioff[w]):int(ioff[w + 1])]
                    )
                    mk_t = sb.tile([128, cpw_max], f32, tag="mk")
                    nc.sync.dma_start(
                        mk_t[:, :nT], mk_in[:, int(csum[w]):int(csum[w + 1])]
                    )
                    m01_t = sb.tile([128, cpw_max, W], bf16, tag="m01")
                    nc.sync.dma_start(
                        m01_t[:, :nT, :],
                        m01_in[:, int(csum[w]) * W:int(csum[w + 1]) * W],
                    )
                    m01t_t = sb.tile([128, cpw_max, CHUNK], bf16, tag="m01t")
                    nc.sync.dma_start(
                        m01t_t[:, :nT, :],
                        m01t_in[:, int(csum[w]) * CHUNK:int(csum[w + 1]) * CHUNK],
                    )
                    a_t = sb.tile([128, cpw_max, F], bf16, tag="a")
                    if nL:
                        nc.gpsimd.dma_gather(
                            a_t[:, :nL, :], xl_fa[:, :], ix_t[:, :nL * 8],
                            nL * CHUNK, nL * CHUNK, F,
                            single_packet=(nL * CHUNK <= 1024),
                        )
                    if nH:
                        nc.gpsimd.dma_gather(
                            a_t[:, nL:nT, :], xl_fb[:, :],
                            ix_t[:, nL * 8:nT * 8], nH * CHUNK, nH * CHUNK, F,
                            single_packet=(nH * CHUNK <= 1024),
                        )
                    xrq_t = sb.tile([128, F], bf16, tag="xrq")
                    nc.sync.dma_start(xrq_t[:W, :], xr_d[w * W:(w + 1) * W, :])
                    out_ps = ps2.tile([W, FQ], f32, space="PSUM", tag="outp")
                    for k0 in range(0, nT, GRP):
                        g = min(GRP, nT - k0)
                        l4 = sb.tile([128, GRP, F], bf16, tag="l4")
                        ap4 = sb.tile([128, GRP, FQ], bf16, tag="ap4")
                        for j in range(g):
                            k = k0 + j
                            b_ps = ps2.tile([128, F], f32, space="PSUM", tag="bps")
                            nc.tensor.matmul(
                                b_ps[:], lhsT=m01t_t[:W, k, :], rhs=xrq_t[:W, :],
                                start=True, stop=False,
                            )
                            nc.tensor.matmul(
                                b_ps[:], lhsT=identb[:], rhs=a_t[:, k, :],
                                start=False, stop=True,
                            )
                            # leaky_relu directly on ACT (Prelu honors alpha)
                            nc.scalar.activation(
                                l4[:, j, :], b_ps[:],
                                mybir.ActivationFunctionType.Prelu, alpha=NEG,
                            )
                        t4 = sb.tile([128, GRP, F], bf16, tag="t4")
                        nc.vector.tensor_tensor(
                            out=t4[:, :g, :], in0=l4[:, :g, :],
                            in1=attg[:, :g, :], op=mybir.AluOpType.mult,
                        )
                        lg4 = sb.tile([128, GRP, H], f32, tag="lg4")
                        nc.vector.reduce_sum(
                            out=lg4[:, :g, :],
                            in_=t4[:, :g, :].rearrange("p k (h c) -> p k h c", h=H),
                            axis=mybir.AxisListType.X,
                        )
                        for j in range(g):
                            k = k0 + j
                            nc.scalar.activation(
                                ap4[:, j, F:FQ], lg4[:, j, :],
                                mybir.ActivationFunctionType.Exp,
                                bias=mk_t[:, k:k + 1], scale=1.0,
                            )
                        nc.vector.tensor_tensor(
                            out=ap4[:, :g, 0:F].rearrange("p k (h c) -> p k h c", h=H),
                            in0=a_t[:, k0:k0 + g, :].rearrange("p k (h c) -> p k h c", h=H),
                            in1=ap4[:, :g, F:FQ].rearrange("p k (h o) -> p k h o", o=1).to_broadcast([128, g, H, C]),
                            op=mybir.AluOpType.mult,
                        )
                        for j in range(g):
                            k = k0 + j
                            nc.tensor.matmul(
                                out_ps[:], lhsT=m01_t[:, k, :], rhs=ap4[:, j, :],
                                start=(k == 0), stop=(k == nT - 1),
                            )
                    # window epilogue
                    rd_t = sb.tile([128, H], f32, tag="rd")
                    nc.vector.reciprocal(rd_t[:W, :], out_ps[:, F:FQ])
                    oh_t = sb.tile([128, H, C], f32, tag="oh")
                    for h in range(H):
                        nc.vector.tensor_scalar(
                            out=oh_t[:W, h, :], in0=out_ps[:, h * C:(h + 1) * C],
                            scalar1=rd_t[:W, h:h + 1], scalar2=0.25,
                            op0=mybir.AluOpType.mult, op1=mybir.AluOpType.mult,
                        )
                    o01 = sb.tile([128, C], f32, tag="o01")
                    nc.vector.tensor_tensor(
                        out=o01[:W, :], in0=oh_t[:W, 0, :], in1=oh_t[:W, 1, :],
                        op=mybir.AluOpType.add,
                    )
                    o23 = sb.tile([128, C], f32, tag="o23")
                    nc.vector.tensor_tensor(
                        out=o23[:W, :], in0=oh_t[:W, 2, :], in1=oh_t[:W, 3, :],
                        op=mybir.AluOpType.add,
                    )
                    o0123 = sb.tile([128, C], f32, tag="o0123")
                    nc.vector.tensor_tensor(
                        out=o0123[:W, :], in0=o01[:W, :], in1=o23[:W, :],
                        op=mybir.AluOpType.add,
                    )
                    nc.vector.tensor_tensor(
                        out=om_all[:W, w, :], in0=o0123[:W, :], in1=biasb_t[:W, :],
                        op=mybir.AluOpType.add,
                    )
                    sq_t = sb.tile([128, C], f32, tag="sq")
                    nc.scalar.square(sq_t[:W, :], om_all[:W, w, :])
                    nc.tensor.matmul(
                        sum_ps[:C, :], lhsT=om_all[:W, w, :], rhs=ones_n[:W, :],
                        start=(w == 0), stop=(w == NW - 1),
                    )
                    nc.tensor.matmul(
                        ssq_ps[:C, :], lhsT=sq_t[:W, :], rhs=ones_n[:W, :],
                        start=(w == 0), stop=(w == NW - 1),
                    )

            # ---- P3: GraphNorm statistics ----
            st_t = sb.tile([128, 2], f32, tag="st")
            nc.vector.tensor_copy(out=st_t[:C, 0:1], in_=sum_ps[:C, :])
            nc.vector.tensor_copy(out=st_t[:C, 1:2], in_=ssq_ps[:C, :])
            nc.sync.dma_start(stats_l[:, :], st_t[:C, :])
            nc.gpsimd.collective_compute(
                "AllReduce", mybir.AluOpType.add,
                replica_groups=[list(range(NCORES))],
                ins=[stats_l.ap().opt()], outs=[stats_g.ap().opt()],
            )
            sg_t = sb.tile([128, 2], f32, tag="sg")
            nc.sync.dma_start(sg_t[:C, :], stats_g[:, :])
            mu_t = sb.tile([128, 1], f32, tag="mu")
            nc.vector.tensor_scalar_mul(mu_t[:C, :], sg_t[:C, 0:1], 1.0 / N)
            msq_t = sb.tile([128, 1], f32, tag="msq")
            nc.vector.tensor_scalar_mul(msq_t[:C, :], sg_t[:C, 1:2], 1.0 / N)
            amu_t = sb.tile([128, 1], f32, tag="amu")
            nc.vector.tensor_tensor(
                out=amu_t[:C, :], in0=gnp_t[:C, 2:3], in1=mu_t[:C, :],
                op=mybir.AluOpType.mult,
            )
            am2_t = sb.tile([128, 1], f32, tag="am2")
            nc.vector.scalar_tensor_tensor(
                out=am2_t[:C, :], in0=mu_t[:C, :], scalar=-2.0, in1=amu_t[:C, :],
                op0=mybir.AluOpType.mult, op1=mybir.AluOpType.add,
            )
            var_t = sb.tile([128, 1], f32, tag="var")
            nc.vector.tensor_tensor(
                out=var_t[:C, :], in0=amu_t[:C, :], in1=am2_t[:C, :],
                op=mybir.AluOpType.mult,
            )
            nc.vector.tensor_tensor(
                out=var_t[:C, :], in0=var_t[:C, :], in1=msq_t[:C, :],
                op=mybir.AluOpType.add,
            )
            nc.vector.tensor_scalar_add(var_t[:C, :], var_t[:C, :], EPS)
            sd_t = sb.tile([128, 1], f32, tag="sd")
            nc.scalar.sqrt(sd_t[:C, :], var_t[:C, :])
            inv_t = sb.tile([128, 1], f32, tag="inv")
            nc.vector.reciprocal(inv_t[:C, :], sd_t[:C, :])
            st2_t = sb.tile([128, 2], f32, tag="st2")
            nc.vector.tensor_tensor(
                out=st2_t[:C, 0:1], in0=gnp_t[:C, 0:1], in1=inv_t[:C, :],
                op=mybir.AluOpType.mult,
            )
            u_t = sb.tile([128, 1], f32, tag="u")
            nc.vector.tensor_tensor(
                out=u_t[:C, :], in0=st2_t[:C, 0:1], in1=amu_t[:C, :],
                op=mybir.AluOpType.mult,
            )
            nc.vector.tensor_tensor(
                out=st2_t[:C, 1:2], in0=gnp_t[:C, 1:2], in1=u_t[:C, :],
                op=mybir.AluOpType.subtract,
            )
            with tc.tile_pool(name="ps3", bufs=1, space="PSUM") as ps3:
                srow_ps = ps3.tile([128, C], f32, space="PSUM", tag="srow")
                nc.tensor.transpose(srow_ps[0:1, :], st2_t[:C, 0:1], ident[:C, :C])
                trow_ps = ps3.tile([128, C], f32, space="PSUM", tag="trow")
                nc.tensor.transpose(trow_ps[0:1, :], st2_t[:C, 1:2], ident[:C, :C])
                srow_t = sb.tile([128, C], f32, tag="srow")
                nc.vector.tensor_copy(out=srow_t[0:1, :], in_=srow_ps[0:1, :])
                trow_t = sb.tile([128, C], f32, tag="trow")
                nc.vector.tensor_copy(out=trow_t[0:1, :], in_=trow_ps[0:1, :])
                sb_ps = ps3.tile([128, C], f32, space="PSUM", tag="sbp")
                nc.tensor.matmul(
                    sb_ps[:W, :], lhsT=ones_r[0:1, :], rhs=srow_t[0:1, :],
                    start=True, stop=True,
                )
                nc.vector.tensor_copy(out=sb_t[:W, :], in_=sb_ps[:W, :])
                tb_ps = ps3.tile([128, C], f32, space="PSUM", tag="tbp")
                nc.tensor.matmul(
                    tb_ps[:W, :], lhsT=ones_r[0:1, :], rhs=trow_t[0:1, :],
                    start=True, stop=True,
                )
                nc.vector.tensor_copy(out=tb_t[:W, :], in_=tb_ps[:W, :])

            # ---- P4: apply GraphNorm affine ----
            for w in range(NW):
                y_t = sb.tile([128, C], f32, tag="y")
                nc.vector.tensor_tensor(
                    out=y_t[:W, :], in0=om_all[:W, w, :], in1=sb_t[:W, :],
                    op=mybir.AluOpType.mult,
                )
                nc.vector.tensor_tensor(
                    out=y_t[:W, :], in0=y_t[:W, :], in1=tb_t[:W, :],
                    op=mybir.AluOpType.add,
                )
                nc.sync.dma_start(
                    out_ext[w * W:(w + 1) * W, :],
                    om_all[:W, w, :] if dbg_om else y_t[:W, :],
                )
    nc.compile()
    return nc


def kernel(X, E, Wl, Wr, att, bias, gn_weight, gn_bias, gn_mean_scale, **kw):
    global LAST_RESULTS
    X = np.asarray(X, np.float32)
    E = np.asarray(E)
    Wl = np.asarray(Wl, np.float32)
    Wr = np.asarray(Wr, np.float32)
    att = np.asarray(att, np.float32)
    bias = np.asarray(bias, np.float32)
    gn_weight = np.asarray(gn_weight, np.float32)
    gn_bias = np.asarray(gn_bias, np.float32)
    gn_mean_scale = np.asarray(gn_mean_scale, np.float32)

    loop = np.arange(N, dtype=np.int64)
    src = np.concatenate([np.asarray(E[0], np.int64), loop])
    dst = np.concatenate([np.asarray(E[1], np.int64), loop])
    cl, ch, per_core = _prep_edges(src, dst)

    attb = np.tile(att.reshape(1, F), (128, 1)).astype(np.float32)
    gnp = np.stack([gn_weight, gn_bias, gn_mean_scale, bias], axis=1).astype(np.float32)

    p0 = per_core[0]
    nc = _build(cl, ch, p0["IDX"].shape[1], p0["MK"].shape[1],
                p0["M01"].shape[1], p0["M01T"].shape[1])

    in_maps = []
    for c in range(NCORES):
        in_maps.append(
            dict(
                XS=np.ascontiguousarray(X[c * NSH:(c + 1) * NSH]),
                WL=Wl, WR=Wr, ATTB=attb, GNP=gnp,
                IDX=per_core[c]["IDX"], MK=per_core[c]["MK"],
                M01=per_core[c]["M01"], M01T=per_core[c]["M01T"],
            )
        )
    trace = bool(kw.get("trace"))
    res = run_bass_kernel_spmd(
        nc, in_maps, core_ids=list(range(NCORES)), trace=trace
    )
    LAST_RESULTS = res
    return np.concatenate([res.results[c]["OUT"] for c in range(NCORES)], axis=0)


# revision 15
# speedup vs baseline: 1.9878x; 1.1315x over previous
"""GATv2Conv (heads=4, concat=False, self-loops) + GraphNorm on 8 TRN2 NeuronCores.

v4 design notes:
- Edges sharded by destination range (6250 dsts/core), processed in 125-dst
  windows, 128-edge chunks, sorted by (window, src-subshard).
- xl is all-gathered as TWO tables (per-core rows [0:3200) and [3200:6250))
  so edge chunks whose sources live in the first table can start gathering
  before the second all-gather lands. Both tables stay under the int16
  gather-index limit.
- Per chunk, PE computes s = xl[src]+xr[dst] in PSUM via a one-hot pick
  matmul (host-precomputed M01T) plus an identity accumulate of the
  gathered rows; ACT applies leaky-relu directly (Prelu, alpha honored,
  unlike the table-fixed Lrelu); DVE multiplies by a replicated att and
  reduces per head; ACT exponentiates with the padding mask as bias.
- One matmul per chunk scatters [a*p || p] through M01 into a [125, 260]
  PSUM tile: softmax numerators and denominators in one accumulation group
  (two interleaved start/stop groups in one PSUM tile corrupt results).
- om (pre-GraphNorm output) stays SBUF-resident; GraphNorm stats via
  AllReduce, then the affine pass writes the output.
"""
import os
import sys

sys.path.insert(0, "/opt/trn_rl_repo")

import ml_dtypes
import numpy as np
from concourse import bacc, mybir, tile
from concourse.bass_utils import run_bass_kernel_spmd
from concourse.masks import make_identity

N = 50000
NCORES = 8
NSH = N // NCORES          # 6250 dst nodes per core
SPL = 3200                 # sub-shard split: rows [0:SPL) -> table A
NA = NCORES * SPL          # 25600 rows in table A
NB = NCORES * (NSH - SPL)  # 24400 rows in table B
IN_F = 256
H = 4
C = 64
F = H * C                  # 256
FQ = F + H                 # 260: scatter rhs = [a*p || p]
W = 125                    # dst window size
NW = NSH // W              # 50 windows per core
NEG = 0.2
EPS = 1e-5
MASKVAL = -100.0           # logit bias for padding edges -> exp == 0
CHUNK = 128
GRP = 4                    # chunks fused per DVE op group

f32 = mybir.dt.float32
bf16 = mybir.dt.bfloat16
i16 = mybir.dt.int16

LAST_RESULTS = None


def _pack_idx(idx: np.ndarray) -> np.ndarray:
    """[n] int -> [128, n//16] int16 gather-index layout (16-partition wrap,
    replicated for the 8 Q7 cores)."""
    n = idx.shape[0]
    pk = np.zeros((16, n // 16), np.int16)
    pk[np.arange(n) % 16, np.arange(n) // 16] = idx.astype(np.int16)
    return np.tile(pk, (8, 1))


def _prep_edges(src: np.ndarray, dst: np.ndarray):
    """Partition/sort/pad edges. Returns (cl, ch, per_core); per_core dicts
    hold IDX (int16), MK (f32 mask), M01/M01T (bf16 one-hot blocks)."""
    src = src.astype(np.int64)
    dst = dst.astype(np.int64)
    core = dst // NSH
    scid = src // NSH                  # source core
    soff = src % NSH                   # offset within source shard
    is_b = soff >= SPL
    tidx = np.where(is_b, scid * (NSH - SPL) + soff - SPL, scid * SPL + soff)
    per_core_raw = []
    nlo = np.zeros((NCORES, NW), np.int64)
    nhi = np.zeros((NCORES, NW), np.int64)
    for c in range(NCORES):
        m = core == c
        ti = tidx[m]
        hb = is_b[m].astype(np.int64)
        d = dst[m] - c * NSH
        win = d // W
        order = np.lexsort((hb, win))
        ti, d, hb = ti[order], d[order], hb[order]
        key = (d // W) * 2 + hb
        cnt = np.bincount(key, minlength=NW * 2).reshape(NW, 2)
        nlo[c] = cnt[:, 0]
        nhi[c] = cnt[:, 1]
        per_core_raw.append((ti, d, np.cumsum(cnt.reshape(-1))))
    cl = np.ceil(nlo.max(axis=0) / CHUNK).astype(int)
    ch = np.ceil(nhi.max(axis=0) / CHUNK).astype(int)

    dcols = np.arange(W, dtype=np.int64)
    per_core = []
    for c in range(NCORES):
        ti, d, cum = per_core_raw[c]
        idx_cols, mk_cols, m01_cols, m01t_cols = [], [], [], []
        for w in range(NW):
            beg_lo = cum[w * 2 - 1] if w * 2 > 0 else 0
            end_lo = cum[w * 2]
            end_hi = cum[w * 2 + 1]
            t_lo, d_lo = ti[beg_lo:end_lo], d[beg_lo:end_lo]
            t_hi, d_hi = ti[end_lo:end_hi], d[end_lo:end_hi]
            nL, nH = cl[w] * CHUNK, ch[w] * CHUNK
            padL = nL - len(t_lo)
            padH = nH - len(t_hi)
            a_lo = np.concatenate([t_lo, np.zeros(padL, np.int64)])
            a_hi = np.concatenate([t_hi, np.zeros(padH, np.int64)])
            rr = np.concatenate(
                [d_lo - w * W, np.zeros(padL, np.int64),
                 d_hi - w * W, np.zeros(padH, np.int64)]
            )
            mm = np.concatenate(
                [np.zeros(len(t_lo), np.float32), np.full(padL, MASKVAL, np.float32),
                 np.zeros(len(t_hi), np.float32), np.full(padH, MASKVAL, np.float32)]
            )
            cols = []
            if nL:
                cols.append(_pack_idx(a_lo))
            if nH:
                cols.append(_pack_idx(a_hi))
            idx_cols.append(np.concatenate(cols, axis=1))
            cpw = cl[w] + ch[w]
            mk_cols.append(mm.reshape(cpw, CHUNK).T.copy())
            rt = rr.reshape(cpw, CHUNK)                      # [k, e]
            m01 = (rt.T[:, :, None] == dcols).astype(ml_dtypes.bfloat16)
            m01_cols.append(m01.reshape(128, cpw * W))
            m01t = (dcols[:, None, None] == rt[None, :, :]).astype(ml_dtypes.bfloat16)
            m01t_pad = np.zeros((128, cpw, CHUNK), ml_dtypes.bfloat16)
            m01t_pad[:W] = m01t
            m01t_cols.append(m01t_pad.reshape(128, cpw * CHUNK))
        per_core.append(
            dict(
                IDX=np.concatenate(idx_cols, axis=1),
                MK=np.concatenate(mk_cols, axis=1).astype(np.float32),
                M01=np.concatenate(m01_cols, axis=1),
                M01T=np.concatenate(m01t_cols, axis=1),
            )
        )
    return cl, ch, per_core


def _build(cl, ch, icols, mkcols, mcols, mtcols):
    """Build the 8-core SPMD graph."""
    nc = bacc.Bacc(None, target_bir_lowering=False, debug=False, num_devices=NCORES)

    xs_in = nc.declare_dram_parameter("XS", [NSH, IN_F], f32, isOutput=False)
    wl_in = nc.declare_dram_parameter("WL", [IN_F, F], f32, isOutput=False)
    wr_in = nc.declare_dram_parameter("WR", [IN_F, F], f32, isOutput=False)
    attb_in = nc.declare_dram_parameter("ATTB", [128, F], f32, isOutput=False)
    gnp_in = nc.declare_dram_parameter("GNP", [C, 4], f32, isOutput=False)
    idx_in = nc.declare_dram_parameter("IDX", [128, icols], i16, isOutput=False)
    mk_in = nc.declare_dram_parameter("MK", [128, mkcols], f32, isOutput=False)
    m01_in = nc.declare_dram_parameter("M01", [128, mcols], bf16, isOutput=False)
    m01t_in = nc.declare_dram_parameter("M01T", [128, mtcols], bf16, isOutput=False)
    out_ext = nc.declare_dram_parameter("OUT", [NSH, C], f32, isOutput=True)

    xl_sha = nc.dram_tensor("xl_sha", [SPL, F], bf16)
    xl_shb = nc.dram_tensor("xl_shb", [NSH - SPL, F], bf16)
    xr_d = nc.dram_tensor("xr_d", [NSH, F], bf16)
    xl_fa = nc.dram_tensor("xl_fa", [NA, F], bf16, addr_space="Shared")
    xl_fb = nc.dram_tensor("xl_fb", [NB, F], bf16, addr_space="Shared")
    stats_l = nc.dram_tensor("stats_l", [C, 2], f32)
    stats_g = nc.dram_tensor("stats_g", [C, 2], f32, addr_space="Shared")

    cpw = [int(cl[w] + ch[w]) for w in range(NW)]
    cpw_max = max(cpw)
    csum = np.concatenate([[0], np.cumsum(cpw)])
    ioff = csum * 8
    dbg_om = bool(int(os.environ.get("DBG_OM", "0")))

    with tile.TileContext(nc) as tc:
        with (
            tc.tile_pool(name="const", bufs=1) as cp,
            tc.tile_pool(name="sb", bufs=2) as sb,
            tc.tile_pool(name="acc", bufs=1, space="PSUM") as accp,
        ):
            ident = cp.tile([128, 128], f32)
            make_identity(nc, ident[:])
            identb = cp.tile([128, 128], bf16)
            nc.vector.tensor_copy(out=identb[:], in_=ident[:])
            wl_t = cp.tile([128, 2, F], bf16)
            wr_t = cp.tile([128, 2, F], bf16)
            wl_f = cp.tile([128, 2, F], f32)
            wr_f = cp.tile([128, 2, F], f32)
            nc.sync.dma_start(wl_f[:], wl_in.ap().rearrange("(s k) n -> k s n", k=128))
            nc.sync.dma_start(wr_f[:], wr_in.ap().rearrange("(s k) n -> k s n", k=128))
            nc.vector.tensor_copy(out=wl_t[:], in_=wl_f[:])
            nc.vector.tensor_copy(out=wr_t[:], in_=wr_f[:])
            # att replicated 4x along free dim for group ops
            attb_f = cp.tile([128, F], f32)
            nc.sync.dma_start(attb_f[:], attb_in[:, :])
            attg = cp.tile([128, GRP, F], bf16)
            for j in range(GRP):
                nc.vector.tensor_copy(out=attg[:, j, :], in_=attb_f[:])
            gnp_t = cp.tile([128, 4], f32)
            nc.sync.dma_start(gnp_t[:C, :], gnp_in[:, :])
            ones_n = cp.tile([128, 1], f32)
            nc.vector.memset(ones_n[:], 1.0)
            ones_r = cp.tile([128, W], f32)
            nc.vector.memset(ones_r[0:1, :], 1.0)
            biasb_t = cp.tile([128, C], f32)
            sb_t = cp.tile([128, C], f32)
            tb_t = cp.tile([128, C], f32)
            om_all = cp.tile([128, NW, C], f32)

            # biasB: [W, C] replicated GAT bias (gnp col 3)
            with tc.tile_pool(name="psi", bufs=1, space="PSUM") as psi:
                brow_ps = psi.tile([128, C], f32, space="PSUM", tag="brow")
                nc.tensor.transpose(brow_ps[0:1, :], gnp_t[:C, 3:4], ident[:C, :C])
                brow_t = cp.tile([128, C], f32)
                nc.vector.tensor_copy(out=brow_t[0:1, :], in_=brow_ps[0:1, :])
                biasb_ps = psi.tile([128, C], f32, space="PSUM", tag="bb")
                nc.tensor.matmul(
                    biasb_ps[:W, :], lhsT=ones_r[0:1, :], rhs=brow_t[0:1, :],
                    start=True, stop=True,
                )
                nc.vector.tensor_copy(out=biasb_t[:W, :], in_=biasb_ps[:W, :])

            # ---- P0: xl/xr transforms (bf16) ----
            ntile = (NSH + 127) // 128
            with tc.tile_pool(name="ps0", bufs=2, space="PSUM") as ps0:
                for i in range(ntile):
                    rows = min(128, NSH - i * 128)
                    x_t = sb.tile([128, IN_F], f32, tag="p0x")
                    nc.sync.dma_start(x_t[:rows, :], xs_in[i * 128:i * 128 + rows, :])
                    xb_t = sb.tile([128, IN_F], bf16, tag="p0xb")
                    nc.vector.tensor_copy(out=xb_t[:rows, :], in_=x_t[:rows, :])
                    xt_sb = sb.tile([128, 2, 128], bf16, tag="p0xt")
                    for hh in range(2):
                        xt_ps = ps0.tile([128, 128], bf16, space="PSUM", tag="p0tp")
                        nc.tensor.transpose(
                            xt_ps[:, :rows], xb_t[:rows, hh * 128:(hh + 1) * 128],
                            identb[:rows, :rows],
                        )
                        nc.vector.tensor_copy(out=xt_sb[:, hh, :rows], in_=xt_ps[:, :rows])
                    if i * 128 < SPL:
                        xl_dram, xl_row = xl_sha, i * 128
                    else:
                        xl_dram, xl_row = xl_shb, i * 128 - SPL
                    for w_t, dram, row0 in ((wl_t, xl_dram, xl_row), (wr_t, xr_d, i * 128)):
                        mm_ps = ps0.tile([128, F], f32, space="PSUM", tag="p0mm")
                        for hh in range(2):
                            nc.tensor.matmul(
                                mm_ps[:rows, :], lhsT=xt_sb[:, hh, :rows],
                                rhs=w_t[:, hh, :], start=(hh == 0), stop=(hh == 1),
                            )
                        o_t = sb.tile([128, F], bf16, tag="p0o")
                        nc.vector.tensor_copy(out=o_t[:rows, :], in_=mm_ps[:rows, :])
                        nc.sync.dma_start(dram[row0:row0 + rows, :], o_t[:rows, :])

            # ---- P1: all-gather xl (two halves; A lands first) ----
            nc.gpsimd.collective_compute(
                "AllGather", mybir.AluOpType.bypass,
                replica_groups=[list(range(NCORES))],
                ins=[xl_sha.ap().opt()], outs=[xl_fa.ap().opt()],
            )
            nc.gpsimd.collective_compute(
                "AllGather", mybir.AluOpType.bypass,
                replica_groups=[list(range(NCORES))],
                ins=[xl_shb.ap().opt()], outs=[xl_fb.ap().opt()],
            )

            # ---- P2: windowed edge processing ----
            sum_ps = accp.tile([128, 1], f32, space="PSUM", tag="accsum")
            ssq_ps = accp.tile([128, 1], f32, space="PSUM", tag="accssq")
            with tc.tile_pool(name="ps2", bufs=2, space="PSUM") as ps2:
                for w in range(NW):
                    nL, nH, nT = int(cl[w]), int(ch[w]), cpw[w]
                    ix_t = sb.tile([128, cpw_max * 8], i16, tag="ix", bufs=3)
                    nc.sync.dma_start(
                        ix_t[:, :nT * 8], idx_in[:, int(ioff[w]):int(ioff[w + 1])]
                    )
                    mk_t = sb.tile([128, cpw_max], f32, tag="mk", bufs=3)
                    nc.sync.dma_start(
                        mk_t[:, :nT], mk_in[:, int(csum[w]):int(csum[w + 1])]
                    )
                    m01_t = sb.tile([128, cpw_max, W], bf16, tag="m01", bufs=3)
                    nc.sync.dma_start(
                        m01_t[:, :nT, :],
                        m01_in[:, int(csum[w]) * W:int(csum[w + 1]) * W],
                    )
                    m01t_t = sb.tile([128, cpw_max, CHUNK], bf16, tag="m01t", bufs=3)
                    nc.sync.dma_start(
                        m01t_t[:, :nT, :],
                        m01t_in[:, int(csum[w]) * CHUNK:int(csum[w + 1]) * CHUNK],
                    )
                    a_t = sb.tile([128, cpw_max, F], bf16, tag="a", bufs=3)
                    if nL:
                        nc.gpsimd.dma_gather(
                            a_t[:, :nL, :], xl_fa[:, :], ix_t[:, :nL * 8],
                            nL * CHUNK, nL * CHUNK, F,
                            single_packet=(nL * CHUNK <= 1024),
                        )
                    if nH:
                        nc.gpsimd.dma_gather(
                            a_t[:, nL:nT, :], xl_fb[:, :],
                            ix_t[:, nL * 8:nT * 8], nH * CHUNK, nH * CHUNK, F,
                            single_packet=(nH * CHUNK <= 1024),
                        )
                    xrq_t = sb.tile([128, F], bf16, tag="xrq", bufs=3)
                    nc.sync.dma_start(xrq_t[:W, :], xr_d[w * W:(w + 1) * W, :])
                    out_ps = ps2.tile([W, FQ], f32, space="PSUM", tag="outp")
                    for k0 in range(0, nT, GRP):
                        g = min(GRP, nT - k0)
                        l4 = sb.tile([128, GRP, F], bf16, tag="l4")
                        ap4 = sb.tile([128, GRP, FQ], bf16, tag="ap4")
                        for j in range(g):
                            k = k0 + j
                            b_ps = ps2.tile([128, F], f32, space="PSUM", tag="bps", bufs=3)
                            nc.tensor.matmul(
                                b_ps[:], lhsT=m01t_t[:W, k, :], rhs=xrq_t[:W, :],
                                start=True, stop=False,
                            )
                            nc.tensor.matmul(
                                b_ps[:], lhsT=identb[:], rhs=a_t[:, k, :],
                                start=False, stop=True,
                            )
                            # leaky_relu directly on ACT (Prelu honors alpha)
                            nc.scalar.activation(
                                l4[:, j, :], b_ps[:],
                                mybir.ActivationFunctionType.Prelu, alpha=NEG,
                            )
                        t4 = sb.tile([128, GRP, F], bf16, tag="t4")
                        nc.vector.tensor_tensor(
                            out=t4[:, :g, :], in0=l4[:, :g, :],
                            in1=attg[:, :g, :], op=mybir.AluOpType.mult,
                        )
                        lg4 = sb.tile([128, GRP, H], f32, tag="lg4")
                        nc.vector.reduce_sum(
                            out=lg4[:, :g, :],
                            in_=t4[:, :g, :].rearrange("p k (h c) -> p k h c", h=H),
                            axis=mybir.AxisListType.X,
                        )
                        for j in range(g):
                            k = k0 + j
                            nc.scalar.activation(
                                ap4[:, j, F:FQ], lg4[:, j, :],
                                mybir.ActivationFunctionType.Exp,
                                bias=mk_t[:, k:k + 1], scale=1.0,
                            )
                        nc.vector.tensor_tensor(
                            out=ap4[:, :g, 0:F].rearrange("p k (h c) -> p k h c", h=H),
                            in0=a_t[:, k0:k0 + g, :].rearrange("p k (h c) -> p k h c", h=H),
                            in1=ap4[:, :g, F:FQ].rearrange("p k (h o) -> p k h o", o=1).to_broadcast([128, g, H, C]),
                            op=mybir.AluOpType.mult,
                        )
                        for j in range(g):
                            k = k0 + j
                            nc.tensor.matmul(
                                out_ps[:], lhsT=m01_t[:, k, :], rhs=ap4[:, j, :],
                                start=(k == 0), stop=(k == nT - 1),
                            )
                    # window epilogue
                    rd_t = sb.tile([128, H], f32, tag="rd")
                    nc.vector.reciprocal(rd_t[:W, :], out_ps[:, F:FQ])
                    oh_t = sb.tile([128, H, C], f32, tag="oh")
                    for h in range(H):
                        nc.vector.tensor_scalar(
                            out=oh_t[:W, h, :], in0=out_ps[:, h * C:(h + 1) * C],
                            scalar1=rd_t[:W, h:h + 1], scalar2=0.25,
                            op0=mybir.AluOpType.mult, op1=mybir.AluOpType.mult,
                        )
                    o01 = sb.tile([128, C], f32, tag="o01")
                    nc.vector.tensor_tensor(
                        out=o01[:W, :], in0=oh_t[:W, 0, :], in1=oh_t[:W, 1, :],
                        op=mybir.AluOpType.add,
                    )
                    o23 = sb.tile([128, C], f32, tag="o23")
                    nc.vector.tensor_tensor(
                        out=o23[:W, :], in0=oh_t[:W, 2, :], in1=oh_t[:W, 3, :],
                        op=mybir.AluOpType.add,
                    )
                    o0123 = sb.tile([128, C], f32, tag="o0123")
                    nc.vector.tensor_tensor(
                        out=o0123[:W, :], in0=o01[:W, :], in1=o23[:W, :],
                        op=mybir.AluOpType.add,
                    )
                    nc.vector.tensor_tensor(
                        out=om_all[:W, w, :], in0=o0123[:W, :], in1=biasb_t[:W, :],
                        op=mybir.AluOpType.add,
                    )
                    sq_t = sb.tile([128, C], f32, tag="sq")
                    nc.scalar.square(sq_t[:W, :], om_all[:W, w, :])
                    nc.tensor.matmul(
                        sum_ps[:C, :], lhsT=om_all[:W, w, :], rhs=ones_n[:W, :],
                        start=(w == 0), stop=(w == NW - 1),
                    )
                    nc.tensor.matmul(
                        ssq_ps[:C, :], lhsT=sq_t[:W, :], rhs=ones_n[:W, :],
                        start=(w == 0), stop=(w == NW - 1),
                    )

            # ---- P3: GraphNorm statistics ----
            st_t = sb.tile([128, 2], f32, tag="st")
            nc.vector.tensor_copy(out=st_t[:C, 0:1], in_=sum_ps[:C, :])
            nc.vector.tensor_copy(out=st_t[:C, 1:2], in_=ssq_ps[:C, :])
            nc.sync.dma_start(stats_l[:, :], st_t[:C, :])
            nc.gpsimd.collective_compute(
                "AllReduce", mybir.AluOpType.add,
                replica_groups=[list(range(NCORES))],
                ins=[stats_l.ap().opt()], outs=[stats_g.ap().opt()],
            )
            sg_t = sb.tile([128, 2], f32, tag="sg")
            nc.sync.dma_start(sg_t[:C, :], stats_g[:, :])
            mu_t = sb.tile([128, 1], f32, tag="mu")
            nc.vector.tensor_scalar_mul(mu_t[:C, :], sg_t[:C, 0:1], 1.0 / N)
            msq_t = sb.tile([128, 1], f32, tag="msq")
            nc.vector.tensor_scalar_mul(msq_t[:C, :], sg_t[:C, 1:2], 1.0 / N)
            amu_t = sb.tile([128, 1], f32, tag="amu")
            nc.vector.tensor_tensor(
                out=amu_t[:C, :], in0=gnp_t[:C, 2:3], in1=mu_t[:C, :],
                op=mybir.AluOpType.mult,
            )
            am2_t = sb.tile([128, 1], f32, tag="am2")
            nc.vector.scalar_tensor_tensor(
                out=am2_t[:C, :], in0=mu_t[:C, :], scalar=-2.0, in1=amu_t[:C, :],
                op0=mybir.AluOpType.mult, op1=mybir.AluOpType.add,
            )
            var_t = sb.tile([128, 1], f32, tag="var")
            nc.vector.tensor_tensor(
                out=var_t[:C, :], in0=amu_t[:C, :], in1=am2_t[:C, :],
                op=mybir.AluOpType.mult,
            )
            nc.vector.tensor_tensor(
                out=var_t[:C, :], in0=var_t[:C, :], in1=msq_t[:C, :],
                op=mybir.AluOpType.add,
            )
            nc.vector.tensor_scalar_add(var_t[:C, :], var_t[:C, :], EPS)
            sd_t = sb.tile([128, 1], f32, tag="sd")
            nc.scalar.sqrt(sd_t[:C, :], var_t[:C, :])
            inv_t = sb.tile([128, 1], f32, tag="inv")
            nc.vector.reciprocal(inv_t[:C, :], sd_t[:C, :])
            st2_t = sb.tile([128, 2], f32, tag="st2")
            nc.vector.tensor_tensor(
                out=st2_t[:C, 0:1], in0=gnp_t[:C, 0:1], in1=inv_t[:C, :],
                op=mybir.AluOpType.mult,
            )
            u_t = sb.tile([128, 1], f32, tag="u")
            nc.vector.tensor_tensor(
                out=u_t[:C, :], in0=st2_t[:C, 0:1], in1=amu_t[:C, :],
                op=mybir.AluOpType.mult,
            )
            nc.vector.tensor_tensor(
                out=st2_t[:C, 1:2], in0=gnp_t[:C, 1:2], in1=u_t[:C, :],
                op=mybir.AluOpType.subtract,
            )
            with tc.tile_pool(name="ps3", bufs=1, space="PSUM") as ps3:
                srow_ps = ps3.tile([128, C], f32, space="PSUM", tag="srow")
                nc.tensor.transpose(srow_ps[0:1, :], st2_t[:C, 0:1], ident[:C, :C])
                trow_ps = ps3.tile([128, C], f32, space="PSUM", tag="trow")
                nc.tensor.transpose(trow_ps[0:1, :], st2_t[:C, 1:2], ident[:C, :C])
                srow_t = sb.tile([128, C], f32, tag="srow")
                nc.vector.tensor_copy(out=srow_t[0:1, :], in_=srow_ps[0:1, :])
                trow_t = sb.tile([128, C], f32, tag="trow")
                nc.vector.tensor_copy(out=trow_t[0:1, :], in_=trow_ps[0:1, :])
                sb_ps = ps3.tile([128, C], f32, space="PSUM", tag="sbp")
                nc.tensor.matmul(
                    sb_ps[:W, :], lhsT=ones_r[0:1, :], rhs=srow_t[0:1, :],
                    start=True, stop=True,
                )
                nc.vector.tensor_copy(out=sb_t[:W, :], in_=sb_ps[:W, :])
                tb_ps = ps3.tile([128, C], f32, space="PSUM", tag="tbp")
                nc.tensor.matmul(
                    tb_ps[:W, :], lhsT=ones_r[0:1, :], rhs=trow_t[0:1, :],
                    start=True, stop=True,
                )
                nc.vector.tensor_copy(out=tb_t[:W, :], in_=tb_ps[:W, :])

            # ---- P4: apply GraphNorm affine (batched) ----
            y_all = sb.tile([128, NW, C], f32, tag="yall", bufs=1)
            nc.vector.tensor_tensor(
                out=y_all[:W, :, :], in0=om_all[:W, :, :],
                in1=sb_t[:W, :].rearrange("p (o c) -> p o c", o=1).to_broadcast([W, NW, C]),
                op=mybir.AluOpType.mult,
            )
            nc.vector.tensor_tensor(
                out=y_all[:W, :, :], in0=y_all[:W, :, :],
                in1=tb_t[:W, :].rearrange("p (o c) -> p o c", o=1).to_broadcast([W, NW, C]),
                op=mybir.AluOpType.add,
            )
            nc.sync.dma_start(
                out_ext.ap().rearrange("(w p) c -> p w c", p=W),
                om_all[:W, :, :] if dbg_om else y_all[:W, :, :],
            )
    nc.compile()
    return nc


def kernel(X, E, Wl, Wr, att, bias, gn_weight, gn_bias, gn_mean_scale, **kw):
    global LAST_RESULTS
    X = np.asarray(X, np.float32)
    E = np.asarray(E)
    Wl = np.asarray(Wl, np.float32)
    Wr = np.asarray(Wr, np.float32)
    att = np.asarray(att, np.float32)
    bias = np.asarray(bias, np.float32)
    gn_weight = np.asarray(gn_weight, np.float32)
    gn_bias = np.asarray(gn_bias, np.float32)
    gn_mean_scale = np.asarray(gn_mean_scale, np.float32)

    loop = np.arange(N, dtype=np.int64)
    src = np.concatenate([np.asarray(E[0], np.int64), loop])
    dst = np.concatenate([np.asarray(E[1], np.int64), loop])
    cl, ch, per_core = _prep_edges(src, dst)

    attb = np.tile(att.reshape(1, F), (128, 1)).astype(np.float32)
    gnp = np.stack([gn_weight, gn_bias, gn_mean_scale, bias], axis=1).astype(np.float32)

    p0 = per_core[0]
    nc = _build(cl, ch, p0["IDX"].shape[1], p0["MK"].shape[1],
                p0["M01"].shape[1], p0["M01T"].shape[1])

    in_maps = []
    for c in range(NCORES):
        in_maps.append(
            dict(
                XS=np.ascontiguousarray(X[c * NSH:(c + 1) * NSH]),
                WL=Wl, WR=Wr, ATTB=attb, GNP=gnp,
                IDX=per_core[c]["IDX"], MK=per_core[c]["MK"],
                M01=per_core[c]["M01"], M01T=per_core[c]["M01T"],
            )
        )
    trace = bool(kw.get("trace"))
    res = run_bass_kernel_spmd(
        nc, in_maps, core_ids=list(range(NCORES)), trace=trace
    )
    LAST_RESULTS = res
    return np.concatenate([res.results[c]["OUT"] for c in range(NCORES)], axis=0)
